# revision 1
# baseline (speedup 1.0000x reference)
"""AttnBlock fusion kernel for Trainium2 (Bass/Tile), 8 NeuronCores.

Reference computation (per batch element b; c=512 channels, hw=1024 spatial):
    h  = GroupNorm(32, c)(x) ; k = Wk h + bk ; v = Wv h + bv
    y_ = GroupNorm(32, c)(y) ; q = Wq y_ + bq
    attn = softmax_j(q^T k / sqrt(c)) ; o = v @ attn^T ; out = x + Wp o + bp

Sharding: pure data parallel over batch (16 batches / 8 cores = 2 each).

Key algebraic moves (exactness notes inline):
  * bk cancels in softmax over j (adds a per-i constant to logits) -> dropped.
  * bv contributes bv * sum_j(attn) = bv exactly -> folded into
    bp' = bp + Wp @ bv on the host.
  * v is produced directly transposed (vT[j,c]) by the projection matmul,
    and attention scores are computed as S[j,i]; no on-chip transposes.
  * softmax denominator: ones[128,128] matmul gives the partition-broadcast
    column sum of exp(S) directly in PSUM.
All matmuls run in float32r (full fp32 storage, 1 cycle/row at N=512).
"""

import math
import os
import sys

import numpy as np

for _p in ("/opt/trn_rl_repo", "/root/.axon_site/_ro/trn_rl_repo"):
    if os.path.isdir(_p) and _p not in sys.path:
        sys.path.append(_p)

import concourse.bass as bass
import concourse.bacc as bacc
import concourse.mybir as mybir
import concourse.tile as tile
from concourse.bass_utils import run_bass_kernel_spmd

F32 = mybir.dt.float32
F32R = mybir.dt.float32r
AF = mybir.ActivationFunctionType
ALU = mybir.AluOpType

B, C, H, W = 16, 512, 32, 32
HW = H * W                  # 1024
NCORES = 8
BPC = B // NCORES           # 2 batches per core
P = 128                     # SBUF partitions
CT = C // P                 # 4 channel tiles
JT = HW // P                # 8 key-position tiles
IBS = 512                   # i-block size (query positions per block)
IB = HW // IBS              # 2 i-blocks
GROUPS = 32
GSIZE = C // GROUPS         # 16 channels per group
EPS = 1e-6
SM_SCALE = float(int(C) ** -0.5)

# prm rows
R_GN_SCALE, R_GN_BIAS, R_GN1_SCALE, R_GN1_BIAS, R_BQ, R_BPP = range(6)


def _r(ap):
    """View an fp32 AP as float32r for the tensor engine."""
    return ap.bitcast(F32R)


def _emit(tc, aps):
    nc = tc.nc
    xs, ys, wq, wk, wv, wp, prm, amat, out = (
        aps["xs"], aps["ys"], aps["wqT"], aps["wkT"], aps["wvT"], aps["wpT"],
        aps["prm"], aps["amat"], aps["out"],
    )

    from contextlib import ExitStack

    with ExitStack() as ctx:
        cpool = ctx.enter_context(tc.tile_pool(name="const", bufs=1))
        wpool = ctx.enter_context(tc.tile_pool(name="w", bufs=1))
        xpool = ctx.enter_context(tc.tile_pool(name="xin", bufs=2))
        ypool = ctx.enter_context(tc.tile_pool(name="yin", bufs=1))
        ynpool = ctx.enter_context(tc.tile_pool(name="yn", bufs=1))
        hpool = ctx.enter_context(tc.tile_pool(name="hb", bufs=1))
        qpool = ctx.enter_context(tc.tile_pool(name="qb", bufs=1))
        kpool = ctx.enter_context(tc.tile_pool(name="kb", bufs=1))
        vpool = ctx.enter_context(tc.tile_pool(name="vb", bufs=1))
        epool = ctx.enter_context(tc.tile_pool(name="eb", bufs=1))
        opool = ctx.enter_context(tc.tile_pool(name="ob", bufs=2))
        rzpool = ctx.enter_context(tc.tile_pool(name="rz", bufs=2))
        outpool = ctx.enter_context(tc.tile_pool(name="outb", bufs=3))
        spool = ctx.enter_context(tc.tile_pool(name="small", bufs=2))
        pmm = ctx.enter_context(tc.tile_pool(name="pmm", bufs=5, space="PSUM"))
        pzb = ctx.enter_context(tc.tile_pool(name="pzb", bufs=2, space="PSUM"))
        pgs = ctx.enter_context(tc.tile_pool(name="pgs", bufs=1, space="PSUM"))

        def load_one(dst_sb, src, chunks):
            """chunks x per-512-col DMAs per c-tile so dependents start early
            and transfers spread across DMA queues."""
            w = HW // chunks
            v = src.rearrange("p (t n) -> p t n", n=HW)
            for t in range(CT):
                for c0 in range(chunks):
                    nc.sync.dma_start(
                        dst_sb[:, t, c0 * w : (c0 + 1) * w],
                        v[:, t, c0 * w : (c0 + 1) * w],
                    )

        def gn_stats_pre(src_sb, uid):
            """DVE-only per-partition statistics: [mean, var, mean^2]."""
            stats = spool.tile([P, CT, 3], F32, tag=f"st{uid}")
            for t in range(CT):
                bns = spool.tile([P, 2, 6], F32, tag=f"bns{uid}")
                for h2 in range(2):
                    nc.vector.bn_stats(
                        bns[:, h2, :], src_sb[:, t, h2 * 512 : (h2 + 1) * 512]
                    )
                nc.vector.bn_aggr(stats[:, t, 0:2], bns[:])
                nc.vector.tensor_tensor(
                    stats[:, t, 2:3], stats[:, t, 0:1], stats[:, t, 0:1],
                    op=ALU.mult,
                )
            return stats

        def gn_stats_post(stats, scale_row, bias_row, uid):
            """Cross-partition group aggregation (one tiny PE matmul) and the
            affine constants a, mb."""
            gps = pgs.tile([P, CT, 3], F32, tag="gs")
            nc.tensor.matmul(gps[:], amat_sb[:], stats[:], start=True, stop=True)
            g = spool.tile([P, CT, 3], F32, tag=f"g{uid}")
            nc.scalar.copy(g[:], gps[:])
            # var_g = E[var] + E[mean^2] - E[mean]^2  (equal-count partitions)
            msq = spool.tile([P, CT], F32, tag=f"msq{uid}")
            nc.vector.tensor_tensor(msq[:], g[:, :, 0], g[:, :, 0], op=ALU.mult)
            var = spool.tile([P, CT], F32, tag=f"var{uid}")
            nc.vector.tensor_tensor(var[:], g[:, :, 1], g[:, :, 2], op=ALU.add)
            nc.vector.tensor_tensor(var[:], var[:], msq[:], op=ALU.subtract)
            nc.vector.tensor_scalar(var[:], var[:], EPS, None, op0=ALU.add)
            # rstd = 1/sqrt(var+eps), Newton-polished
            std = spool.tile([P, CT], F32, tag=f"std{uid}")
            nc.scalar.activation(std[:], var[:], AF.Sqrt)
            r0 = spool.tile([P, CT], F32, tag=f"r0{uid}")
            nc.vector.reciprocal(r0[:], std[:])
            t7 = spool.tile([P, CT], F32, tag=f"t7{uid}")
            nc.vector.tensor_tensor(t7[:], r0[:], r0[:], op=ALU.mult)
            nc.vector.tensor_tensor(t7[:], var[:], t7[:], op=ALU.mult)
            nc.vector.tensor_scalar(t7[:], t7[:], -0.5, 1.5, op0=ALU.mult, op1=ALU.add)
            rstd = spool.tile([P, CT], F32, tag=f"rs{uid}")
            nc.vector.tensor_tensor(rstd[:], r0[:], t7[:], op=ALU.mult)
            # a = rstd*gamma ; mb = beta - mean*a
            a = spool.tile([P, CT], F32, tag=f"a{uid}")
            nc.vector.tensor_tensor(a[:], rstd[:], prm_sb[:, scale_row, :], op=ALU.mult)
            mb = spool.tile([P, CT], F32, tag=f"mb{uid}")
            nc.vector.tensor_tensor(mb[:], g[:, :, 0], a[:], op=ALU.mult)
            nc.vector.tensor_tensor(mb[:], prm_sb[:, bias_row, :], mb[:], op=ALU.subtract)
            return a, mb

        def gn_apply(src_sb, dst_sb, st, engine="dve"):
            a, mb = st
            for t in range(CT):
                if engine == "act":
                    nc.scalar.activation(
                        dst_sb[:, t, :], src_sb[:, t, :], AF.Identity,
                        bias=mb[:, t : t + 1], scale=a[:, t : t + 1],
                    )
                else:
                    nc.vector.tensor_scalar(
                        dst_sb[:, t, :], src_sb[:, t, :],
                        a[:, t : t + 1], mb[:, t : t + 1],
                        op0=ALU.mult, op1=ALU.add,
                    )

        # ---- prologue: x first (gates everything), then wk, y, rest ----
        x_sb = xpool.tile([P, CT, HW], F32, tag="x")
        load_one(x_sb, xs[0], chunks=1)
        prm_sb = cpool.tile([P, 6, CT], F32)
        nc.sync.dma_start(prm_sb[:], prm.rearrange("p (q t) -> p q t", t=CT))
        amat_sb = cpool.tile([P, P], F32)
        nc.sync.dma_start(amat_sb[:], amat[:])
        w_sb = {}

        def loadw(name, ap):
            t = wpool.tile([P, CT, C], F32R, tag=name)
            nc.gpsimd.dma_start(t[:], ap.rearrange("p (t o) -> p t o", o=C))
            w_sb[name] = t

        loadw("wk", wk)
        y_sb = ypool.tile([P, CT, HW], F32, tag="y")
        load_one(y_sb, ys[0], chunks=1)
        ones_mat = cpool.tile([P, P], F32R)
        nc.sync.dma_start(ones_mat[:], aps["ones"][:])
        loadw("wv", wv)
        loadw("wq", wq)
        loadw("wp", wp)

        stats_x = gn_stats_pre(x_sb, uid="x0")
        stats_y = gn_stats_pre(y_sb, uid="y0")
        st = (gn_stats_post(stats_x, R_GN_SCALE, R_GN_BIAS, uid="x0"),
              gn_stats_post(stats_y, R_GN1_SCALE, R_GN1_BIAS, uid="y0"))
        nxt = {}

        for b in range(BPC):
            outv = out[b].rearrange("p (t n) -> p t n", n=HW)

            xcur, ycur = x_sb, y_sb
            if nxt:
                h_sb, yn_sb = nxt.pop("h"), nxt.pop("yn")
            else:
                st_x, st_y = st
                h_sb = hpool.tile([P, CT, HW], F32R, tag="h")
                gn_apply(xcur, h_sb, st_x)
                yn_sb = ynpool.tile([P, CT, HW], F32R, tag="yn")
                gn_apply(ycur, yn_sb, st_y)

            # ---- k = Wk h  (k[c_out, i]) ----
            k_sb = kpool.tile([P, CT, HW], F32R, tag="k")
            for mt in range(CT):
                for nh in range(IB):
                    ps = pmm.tile([P, IBS], F32, tag="ps")
                    for kt in range(CT):
                        nc.tensor.matmul(
                            ps[:],
                            w_sb["wk"][:, kt, mt * P : (mt + 1) * P],
                            h_sb[:, kt, nh * IBS : (nh + 1) * IBS],
                            start=(kt == 0), stop=(kt == CT - 1),
                        )
                    nc.vector.tensor_copy(k_sb[:, mt, nh * IBS : (nh + 1) * IBS], ps[:])

            # ---- vT[j, c_out] = h^T WvT ----
            vT_sb = vpool.tile([P, JT, C], F32R, tag="vT")
            for jt in range(JT):
                ps = pmm.tile([P, C], F32, tag="ps")
                for kt in range(CT):
                    nc.tensor.matmul(
                        ps[:],
                        h_sb[:, kt, jt * P : (jt + 1) * P],
                        w_sb["wv"][:, kt, :],
                        start=(kt == 0), stop=(kt == CT - 1),
                    )
                nc.scalar.copy(vT_sb[:, jt, :], ps[:])

            # ---- q = Wq y_ + bq ----
            q_sb = qpool.tile([P, CT, HW], F32R, tag="q")
            for mt in range(CT):
                for nh in range(IB):
                    ps = pmm.tile([P, IBS], F32, tag="ps")
                    for kt in range(CT):
                        nc.tensor.matmul(
                            ps[:],
                            w_sb["wq"][:, kt, mt * P : (mt + 1) * P],
                            yn_sb[:, kt, nh * IBS : (nh + 1) * IBS],
                            start=(kt == 0), stop=(kt == CT - 1),
                        )
                    nc.vector.tensor_scalar(
                        q_sb[:, mt, nh * IBS : (nh + 1) * IBS], ps[:],
                        prm_sb[:, R_BQ, mt : mt + 1], None, op0=ALU.add,
                    )

            # prefetch next batch + its DVE-only stats; the tiny stats
            # matmul is deferred to the next iteration so it never blocks
            # this batch's attention in the in-order PE queue.
            if b + 1 < BPC:
                x_sb = xpool.tile([P, CT, HW], F32, tag="x")
                load_one(x_sb, xs[b + 1], chunks=1)
                y_sb = ypool.tile([P, CT, HW], F32, tag="y")
                load_one(y_sb, ys[b + 1], chunks=1)
                pend = {"sx": gn_stats_pre(x_sb, uid=f"x{b+1}"),
                        "sy": gn_stats_pre(y_sb, uid=f"y{b+1}")}
            # prefetch next batch's inputs; their DVE-only stats run in
            # attention-phase DVE gaps
            if b + 1 < BPC:
                xn_t = xpool.tile([P, CT, HW], F32, tag="x")
                load_one(xn_t, xs[b + 1], chunks=1)
                yn_t = ypool.tile([P, CT, HW], F32, tag="y")
                load_one(yn_t, ys[b + 1], chunks=1)
                pend = {"sx": gn_stats_pre(xn_t, uid=f"x{b+1}"),
                        "sy": gn_stats_pre(yn_t, uid=f"y{b+1}")}

            # ---- attention; r-projection pipelined one i-block behind ----
            rdefer = []

            def emit_r(ib, mts=range(CT)):
                isl2 = slice(ib * IBS, (ib + 1) * IBS)
                o0_p = rdefer[0]
                for mt in mts:
                    ps = pmm.tile([P, IBS], F32, tag="ps")
                    for ct in range(CT):
                        nc.tensor.matmul(
                            ps[:],
                            w_sb["wp"][:, ct, mt * P : (mt + 1) * P],
                            o0_p[:, ct, :],
                            start=(ct == 0), stop=(ct == CT - 1),
                        )
                    ot = outpool.tile([P, IBS], F32, tag="ot")
                    nc.vector.scalar_tensor_tensor(
                        ot[:], ps[:], prm_sb[:, R_BPP, mt : mt + 1],
                        xcur[:, mt, isl2], op0=ALU.add, op1=ALU.add,
                    )
                    half = IBS // 2
                    for hh in range(2):
                        osl = slice(ib * IBS + hh * half, ib * IBS + (hh + 1) * half)
                        nc.sync.dma_start(outv[:, mt, osl], ot[:, hh * half:(hh + 1) * half])

            for ib in range(IB):
                isl = slice(ib * IBS, (ib + 1) * IBS)
                e_sb = epool.tile([P, JT, IBS], F32R, tag="e")
                zb = pzb.tile([P, IBS], F32, tag="zb")
                for jt in range(JT):
                    ps = pmm.tile([P, IBS], F32, tag="ps")
                    for kt in range(CT):
                        nc.tensor.matmul(
                            ps[:],
                            k_sb[:, kt, jt * P : (jt + 1) * P],
                            q_sb[:, kt, isl],
                            start=(kt == 0), stop=(kt == CT - 1),
                        )
                    # E = exp(S / sqrt(c)); logits are O(1), no max needed
                    nc.scalar.activation(e_sb[:, jt, :], ps[:], AF.Exp, scale=SM_SCALE)
                    # Z[p, i] = sum_j E[j, i] for every p (ones matmul broadcast)
                    nc.tensor.matmul(
                        zb[:], ones_mat[:], e_sb[:, jt, :],
                        start=(jt == 0), stop=(jt == JT - 1),
                    )
                rzb = rzpool.tile([P, IBS], F32, tag="rzb")
                nc.vector.reciprocal(rzb[:], zb[:])

                o0_sb = opool.tile([P, CT, IBS], F32R, tag="o0")
                for ct in range(CT):
                    ps = pmm.tile([P, IBS], F32, tag="ps")
                    for jt in range(JT):
                        nc.tensor.matmul(
                            ps[:],
                            vT_sb[:, jt, ct * P : (ct + 1) * P],
                            e_sb[:, jt, :],
                            start=(jt == 0), stop=(jt == JT - 1),
                        )
                    nc.vector.tensor_tensor(o0_sb[:, ct, :], ps[:], rzb[:], op=ALU.mult)
                rdefer.append(o0_sb)
                if ib > 0:
                    emit_r(ib - 1)
                    rdefer.pop(0)
            # final r-group split around the next batch's GroupNorm chain so
            # its k-matmuls are unblocked the moment this batch finishes
            if b + 1 < BPC:
                emit_r(IB - 1, mts=(0, 1))
                st_xn = gn_stats_post(pend["sx"], R_GN_SCALE, R_GN_BIAS,
                                      uid=f"x{b+1}")
                st_yn = gn_stats_post(pend["sy"], R_GN1_SCALE, R_GN1_BIAS,
                                      uid=f"y{b+1}")
                h_n = hpool.tile([P, CT, HW], F32R, tag="h")
                gn_apply(xn_t, h_n, st_xn, engine="act")
                yn_n = ynpool.tile([P, CT, HW], F32R, tag="yn")
                gn_apply(yn_t, yn_n, st_yn, engine="act")
                nxt = {"h": h_n, "yn": yn_n}
                emit_r(IB - 1, mts=(2, 3))
                x_sb, y_sb = xn_t, yn_t
            else:
                emit_r(IB - 1)


_CACHE = {}


def _build():
    if "nc" in _CACHE:
        return _CACHE["nc"]
    nc = bacc.Bacc("TRN2", target_bir_lowering=False, debug=False)
    aps = {
        "xs": nc.dram_tensor("xs", [BPC, P, CT * HW], F32, kind="ExternalInput").ap(),
        "ys": nc.dram_tensor("ys", [BPC, P, CT * HW], F32, kind="ExternalInput").ap(),
        "wqT": nc.dram_tensor("wqT", [P, CT * C], F32R, kind="ExternalInput").ap(),
        "wkT": nc.dram_tensor("wkT", [P, CT * C], F32R, kind="ExternalInput").ap(),
        "wvT": nc.dram_tensor("wvT", [P, CT * C], F32R, kind="ExternalInput").ap(),
        "wpT": nc.dram_tensor("wpT", [P, CT * C], F32R, kind="ExternalInput").ap(),
        "prm": nc.dram_tensor("prm", [P, 6 * CT], F32, kind="ExternalInput").ap(),
        "amat": nc.dram_tensor("amat", [P, P], F32, kind="ExternalInput").ap(),
        "ones": nc.dram_tensor("ones", [P, P], F32R, kind="ExternalInput").ap(),
        "out": nc.dram_tensor("out", [BPC, P, CT * HW], F32, kind="ExternalOutput").ap(),
    }
    with tile.TileContext(nc) as tc:
        _emit(tc, aps)
    nc.compile()
    _CACHE["nc"] = nc
    return nc


def _pack_chw(a):
    """[*, C, HW] -> [*, P, CT*HW] matching SBUF layout c = t*128 + p."""
    lead = a.shape[:-2]
    a = a.reshape(*lead, CT, P, HW)
    a = np.moveaxis(a, -3, -2)          # [..., P, CT, HW]
    return np.ascontiguousarray(a.reshape(*lead, P, CT * HW))


def _unpack_chw(a):
    """[*, P, CT*HW] -> [*, C, HW]."""
    lead = a.shape[:-2]
    a = a.reshape(*lead, P, CT, HW)
    a = np.moveaxis(a, -2, -3)          # [..., CT, P, HW]
    return np.ascontiguousarray(a.reshape(*lead, CT * P, HW))


def _round_fp32r(a):
    """Round fp32 to the PE's fp32r format: 1+8+11 bits, low 12 zeroed."""
    u = np.ascontiguousarray(a, dtype=np.float32).view(np.uint32)
    u = u + np.uint32(0x7FF) + ((u >> np.uint32(12)) & np.uint32(1))
    u = u & np.uint32(0xFFFFF000)
    return u.view(np.float32)


def _host_inputs(x, y, norm_scale, norm_bias, norm1_scale, norm1_bias,
                 wq, bq, wk, bk, wv, bv, wp, bp):
    f = lambda a: np.ascontiguousarray(np.asarray(a, dtype=np.float32))
    x = f(x).reshape(B, C, HW)
    y = f(y).reshape(B, C, HW)
    wq, wk, wv, wp = f(wq), f(wk), f(wv), f(wp)
    # bk cancels in softmax; bv folds into bp' because softmax rows sum to 1
    bpp = f(bp) + wp @ f(bv)
    prm = np.stack([f(norm_scale), f(norm_bias), f(norm1_scale), f(norm1_bias),
                    f(bq), bpp]).astype(np.float32)
    # [6, C] -> [P, 6*CT] matching prm_sb[p, q, t]
    prm = np.ascontiguousarray(
        prm.reshape(6, CT, P).transpose(2, 0, 1).reshape(P, 6 * CT)
    )
    amat = np.zeros((P, P), np.float32)
    for g in range(P // GSIZE):
        amat[g * GSIZE : (g + 1) * GSIZE, g * GSIZE : (g + 1) * GSIZE] = 1.0 / GSIZE
    def packw(w):
        # wT [c_in, c_out] -> [P, CT*C] matching w_sb[p, kt, o]
        wT = _round_fp32r(w.T)
        return np.ascontiguousarray(
            wT.reshape(CT, P, C).transpose(1, 0, 2).reshape(P, CT * C)
        )

    shared = {
        "wqT": packw(wq), "wkT": packw(wk), "wvT": packw(wv), "wpT": packw(wp),
        "prm": prm, "amat": amat, "ones": np.ones((P, P), np.float32),
    }
    in_maps = []
    for core in range(NCORES):
        sl = slice(core * BPC, (core + 1) * BPC)
        in_maps.append({
            "xs": _pack_chw(x[sl]),
            "ys": _pack_chw(y[sl]),
            **shared,
        })
    return in_maps


def _run(in_maps, trace=False):
    nc = _build()
    res = run_bass_kernel_spmd(
        nc, in_maps, core_ids=list(range(NCORES)), trace=trace
    )
    out = np.concatenate(
        [_unpack_chw(res.results[i]["out"]) for i in range(NCORES)], axis=0
    ).reshape(B, C, H, W)
    return out, res


def kernel(**inputs):
    in_maps = _host_inputs(**inputs)
    out, _ = _run(in_maps, trace=False)
    return out



# revision 10
# speedup vs baseline: 1.3688x; 1.3688x over previous
"""AttnBlock fusion kernel for Trainium2 (Bass/Tile), 8 NeuronCores.

Reference computation (per batch element b; c=512 channels, hw=1024 spatial):
    h  = GroupNorm(32, c)(x) ; k = Wk h + bk ; v = Wv h + bv
    y_ = GroupNorm(32, c)(y) ; q = Wq y_ + bq
    attn = softmax_j(q^T k / sqrt(c)) ; o = v @ attn^T ; out = x + Wp o + bp

Sharding: pure data parallel over batch (16 batches / 8 cores = 2 each).

Algebraic folds (host side, exact):
  * S = q^T k = y_^T (Wq^T Wk) h  -> A := Wq^T Wk precomputed; the q and k
    projections disappear (one matmul t = A h replaces both).
  * Wp (v @ P) = (Wp Wv) h @ P    -> Bm := Wp Wv precomputed; the v and
    proj_out projections disappear (u = Bm h replaces both).
  * bk adds a per-i constant to logits -> cancels in softmax.
  * bv contributes Wp bv exactly (softmax rows sum to 1) -> bp' = bp + Wp bv.
  * bq (zero in practice) handled by a compiled-in logit-bias path.

Precision scheme (validated vs reference: rel_l2 ~ 5e-3, gate 2e-2):
  * All five big matmuls run fp8(e4m3) with MatmulPerfMode.DoubleRow:
    2 contraction tiles per pass = 2x throughput over fp32r/bf16.
  * A, Bm scaled by 16 so t = A h and u = Bm h land in e4m3 range (+-240);
    1/16 is folded into the exp scale (t side) and into ones=16 for the
    Z row-sum matmul (u side, via 1/Z).
  * E = exp(s S - 3): the -3 shift is softmax-invariant and keeps
    max(E) ~ 31 < 240 so no fp8 overflow-to-inf.
  * x, y stored bf16 on chip (stats + gn + residual), accumulation fp32.
"""

import math
import os
import sys
from contextlib import ExitStack

import numpy as np
import ml_dtypes

for _p in ("/opt/trn_rl_repo", "/root/.axon_site/_ro/trn_rl_repo"):
    if os.path.isdir(_p) and _p not in sys.path:
        sys.path.append(_p)

import concourse.bass as bass
import concourse.bacc as bacc
import concourse.mybir as mybir
import concourse.tile as tile
from concourse.bass_utils import run_bass_kernel_spmd

F32 = mybir.dt.float32
BF16 = mybir.dt.bfloat16
F8 = mybir.dt.float8e4
U8 = mybir.dt.uint8
U16 = mybir.dt.uint16
AF = mybir.ActivationFunctionType
ALU = mybir.AluOpType
DR = mybir.MatmulPerfMode.DoubleRow
AX = mybir.AxisListType

B, C, H, W = 16, 512, 32, 32
HW = H * W                  # 1024
NCORES = 8
BPC = B // NCORES           # 2 batches per core
P = 128                     # SBUF partitions
CT = C // P                 # 4 channel tiles
JT = HW // P                # 8 key-position tiles
IBS = 512                   # query positions per i-block
IB = HW // IBS              # 2 i-blocks
GROUPS = 32
GSIZE = C // GROUPS         # 16 channels per group
EPS = 1e-6
SM = float(C) ** -0.5
SA = 16.0                   # scale folded into A
SB = 16.0                   # scale folded into Bm (and into ones for Z)
EXPS = SM / SA
EXPB = -3.0                 # softmax-invariant logit shift, keeps E < 240

# prm rows
R_GXS, R_GXB, R_GYS, R_GYB, R_BPP = range(5)

NPF8 = ml_dtypes.float8_e4m3   # IEEE e4m3 (bias 7, max 240) == TRN FP8_EXP4
NPBF16 = ml_dtypes.bfloat16


def _emit(tc, aps, has_bq):
    nc = tc.nc
    xs, ys, out = aps["xs"], aps["ys"], aps["out"]

    with ExitStack() as ctx:
        cpool = ctx.enter_context(tc.tile_pool(name="const", bufs=1))
        wpool = ctx.enter_context(tc.tile_pool(name="w", bufs=1))
        xpool = ctx.enter_context(tc.tile_pool(name="xin", bufs=2))
        ypool = ctx.enter_context(tc.tile_pool(name="yin", bufs=2))
        hpool = ctx.enter_context(tc.tile_pool(name="hb", bufs=2))
        ynpool = ctx.enter_context(tc.tile_pool(name="ynb", bufs=2))
        tpool = ctx.enter_context(tc.tile_pool(name="tb", bufs=2))
        upool = ctx.enter_context(tc.tile_pool(name="ub", bufs=2))
        epool = ctx.enter_context(tc.tile_pool(name="eb", bufs=2))
        jkpool = ctx.enter_context(tc.tile_pool(name="jk", bufs=2))
        stpool = ctx.enter_context(tc.tile_pool(name="st", bufs=2))
        smpool = ctx.enter_context(tc.tile_pool(name="sm", bufs=3))
        ompool = ctx.enter_context(tc.tile_pool(name="om", bufs=2))
        outpool = ctx.enter_context(tc.tile_pool(name="outb", bufs=3))
        rzpool = ctx.enter_context(tc.tile_pool(name="rz", bufs=2))
        pspool = ctx.enter_context(tc.tile_pool(name="ps", bufs=3, space="PSUM"))
        zpool = ctx.enter_context(tc.tile_pool(name="z", bufs=1, space="PSUM"))
        opool = ctx.enter_context(tc.tile_pool(name="o", bufs=4, space="PSUM"))

        # ---- constants / weights ----
        prm_sb = cpool.tile([P, 5, CT], F32)
        nc.gpsimd.dma_start(prm_sb[:], aps["prm"].rearrange("p (q t) -> p q t", t=CT))
        amat_sb = cpool.tile([P, P], F32)
        nc.gpsimd.dma_start(amat_sb[:], aps["amat"][:])
        ones_sb = cpool.tile([P, 2, P], F8)
        nc.vector.memset(ones_sb[:], SB)
        expb_sb = cpool.tile([P, 1], F32)
        nc.vector.memset(expb_sb[:], EXPB)
        A_sb = wpool.tile([P, 2, 2, C], F8)
        nc.gpsimd.dma_start(
            A_sb[:], aps["A"].rearrange("p (a b o) -> p a b o", a=2, b=2).bitcast(F8)
        )
        Bm_sb = wpool.tile([P, 2, 2, C], F8)
        nc.gpsimd.dma_start(
            Bm_sb[:], aps["Bm"].rearrange("p (a b o) -> p a b o", a=2, b=2).bitcast(F8)
        )
        if has_bq:
            g_sb = cpool.tile([P, CT], F8)
            nc.gpsimd.dma_start(g_sb[:], aps["gv"].bitcast(F8))

        def load_xy(b):
            x_sb = xpool.tile([P, CT, HW], BF16, tag="x")
            v = xs[b].rearrange("p (t n) -> p t n", n=HW).bitcast(BF16)
            for t in range(CT):
                nc.sync.dma_start(x_sb[:, t, :], v[:, t, :])
            y_sb = ypool.tile([P, CT, HW], BF16, tag="y")
            v = ys[b].rearrange("p (t n) -> p t n", n=HW).bitcast(BF16)
            for t in range(CT):
                nc.sync.dma_start(y_sb[:, t, :], v[:, t, :])
            return x_sb, y_sb

        def stats_pre(src, st, u, uid):
            """DVE: per-channel sum and sumsq -> st[:, u, {0,1}, :]."""
            nc.vector.tensor_reduce(st[:, u, 0, :], src[:], axis=AX.X, op=ALU.add)
            jk = jkpool.tile([P, CT, HW], BF16, tag="jk")
            for t in range(CT):
                nc.vector.tensor_tensor(
                    jk[:, t, :], src[:, t, :], src[:, t, :], op=ALU.mult
                )
            nc.vector.tensor_reduce(st[:, u, 1, :], jk[:], axis=AX.X, op=ALU.add)

        def stats_mm(st, u, uid):
            """PE: group aggregation -> gps[:, {mean, E[x^2]}, ct] (x 1/16384)."""
            gt = pspool.tile([P, IBS], F32, tag="ps", name="gps")
            gps = gt[:, 0:8]
            nc.tensor.matmul(gps, amat_sb[:], st[:, u, :, :], start=True, stop=True)
            return gps

        def stats_post(gps, scale_row, bias_row, uid):
            """DVE small math: a = rstd*gamma, mb = beta - mean*a  [P, CT]."""
            g = smpool.tile([P, 2, CT], F32, tag=f"g{uid}")
            nc.vector.tensor_copy(g[:], gps)
            msq = smpool.tile([P, CT], F32, tag=f"ms{uid}")
            nc.vector.tensor_tensor(msq[:], g[:, 0, :], g[:, 0, :], op=ALU.mult)
            var = smpool.tile([P, CT], F32, tag=f"va{uid}")
            nc.vector.tensor_tensor(var[:], g[:, 1, :], msq[:], op=ALU.subtract)
            nc.vector.tensor_scalar(var[:], var[:], EPS, None, op0=ALU.add)
            std = smpool.tile([P, CT], F32, tag=f"sd{uid}")
            nc.scalar.activation(std[:], var[:], AF.Sqrt)
            r0 = smpool.tile([P, CT], F32, tag=f"r0{uid}")
            nc.vector.reciprocal(r0[:], std[:])
            t7 = smpool.tile([P, CT], F32, tag=f"t7{uid}")
            nc.vector.tensor_tensor(t7[:], r0[:], r0[:], op=ALU.mult)
            nc.vector.tensor_tensor(t7[:], var[:], t7[:], op=ALU.mult)
            nc.vector.tensor_scalar(t7[:], t7[:], -0.5, 1.5, op0=ALU.mult, op1=ALU.add)
            rstd = smpool.tile([P, CT], F32, tag=f"rs{uid}")
            nc.vector.tensor_tensor(rstd[:], r0[:], t7[:], op=ALU.mult)
            a = smpool.tile([P, CT], F32, tag=f"a{uid}")
            nc.vector.tensor_tensor(a[:], rstd[:], prm_sb[:, scale_row, :], op=ALU.mult)
            mb = smpool.tile([P, CT], F32, tag=f"mb{uid}")
            nc.vector.tensor_tensor(mb[:], g[:, 0, :], a[:], op=ALU.mult)
            nc.vector.tensor_tensor(mb[:], prm_sb[:, bias_row, :], mb[:], op=ALU.subtract)
            return a, mb

        def gn_apply(src, pool, tag, st8):
            a, mb = st8
            d = pool.tile([P, CT, HW], F8, tag=tag)
            for t in range(CT):
                nc.vector.tensor_scalar(
                    d[:, t, :], src[:, t, :], a[:, t : t + 1], mb[:, t : t + 1],
                    op0=ALU.mult, op1=ALU.add,
                )
            return d

        # ---- prologue: batch 0 GN fully ----
        x_sb, y_sb = load_xy(0)
        st0 = stpool.tile([P, 2, 2, CT], F32, tag="st")
        stats_pre(x_sb, st0, 0, "x0")
        gx = stats_mm(st0, 0, "x0")
        h_sb = gn_apply(x_sb, hpool, "h", stats_post(gx, R_GXS, R_GXB, "x0"))
        stats_pre(y_sb, st0, 1, "y0")
        gy = stats_mm(st0, 1, "y0")
        yn_sb = gn_apply(y_sb, ynpool, "yn", stats_post(gy, R_GYS, R_GYB, "y0"))

        pend = None
        for b in range(BPC):
            outv = out[b].rearrange("p (t n) -> p t n", n=HW)

            # ---- P1a: t = A h  (t[cy, j], fp8) ----
            t_sb = tpool.tile([P, CT, HW], F8, tag="t")
            for nh in range(IB):
                for mt in range(CT):
                    ps = pspool.tile([P, IBS], F32, tag="ps")
                    for kp in range(2):
                        nc.tensor.matmul(
                            ps[:],
                            A_sb[:, kp, :, mt * P : (mt + 1) * P],
                            h_sb[:, 2 * kp : 2 * kp + 2, nh * IBS : (nh + 1) * IBS],
                            start=(kp == 0), stop=(kp == 1), perf_mode=DR,
                        )
                    nc.vector.tensor_copy(
                        t_sb[:, mt, nh * IBS : (nh + 1) * IBS], ps[:]
                    )

            # finish previous iteration's deferred y-side GN (for this batch's
            # successor it is emitted below; for b>0 `pend` carries batch b's
            # y-side which was already resolved last iteration)
            if pend is not None:
                gyn = stats_mm(pend["st"], 1, f"y{b}")
                yn_sb = gn_apply(
                    pend["y"], ynpool, "yn", stats_post(gyn, R_GYS, R_GYB, f"y{b}")
                )
                pend = None

            # ---- P1b: uT = h^T Bm^T  (uT[j, co], fp8) ----
            uT_sb = upool.tile([P, JT, C], F8, tag="u")
            for jt in range(JT):
                ps = pspool.tile([P, C], F32, tag="ps")
                for kp in range(2):
                    nc.tensor.matmul(
                        ps[:],
                        h_sb[:, 2 * kp : 2 * kp + 2, jt * P : (jt + 1) * P],
                        Bm_sb[:, kp, :, :],
                        start=(kp == 0), stop=(kp == 1), perf_mode=DR,
                    )
                nc.vector.tensor_copy(uT_sb[:, jt, :], ps[:])

            # optional bq logit bias: r[j] = g^T h, bias = SM*r + EXPB
            if has_bq:
                rps = zpool.tile([P, JT], F32, tag="z", name="rb")
                for jt in range(JT):
                    for kt in range(CT):
                        nc.tensor.matmul(
                            rps[:, jt : jt + 1],
                            h_sb[:, kt, jt * P : (jt + 1) * P],
                            g_sb[:, kt : kt + 1],
                            start=(kt == 0), stop=(kt == CT - 1),
                        )
                bias_sb = smpool.tile([P, JT], F32, tag="bia")
                nc.vector.tensor_scalar(
                    bias_sb[:], rps[:], SM, EXPB, op0=ALU.mult, op1=ALU.add
                )

            # prefetch next batch inputs
            if b + 1 < BPC:
                xn_sb, ynx_sb = load_xy(b + 1)
                stn = stpool.tile([P, 2, 2, CT], F32, tag="st")

            # ---- attention ----
            e = [
                epool.tile([P, JT, IBS], F8, tag=f"e{ib}", name=f"e{ib}")
                for ib in range(IB)
            ]
            zps = {}

            def S_group(ib, jt):
                ps = pspool.tile([P, IBS], F32, tag="ps")
                for kp in range(2):
                    nc.tensor.matmul(
                        ps[:],
                        t_sb[:, 2 * kp : 2 * kp + 2, jt * P : (jt + 1) * P],
                        yn_sb[:, 2 * kp : 2 * kp + 2, ib * IBS : (ib + 1) * IBS],
                        start=(kp == 0), stop=(kp == 1), perf_mode=DR,
                    )
                bias = bias_sb[:, jt : jt + 1] if has_bq else expb_sb[:]
                nc.scalar.activation(
                    e[ib][:, jt, :], ps[:], AF.Exp, bias=bias, scale=EXPS
                )

            def Z_mm(ib, pr):
                if pr == 0:
                    zps[ib] = zpool.tile([P, IBS], F32, tag="z", name="z")
                nc.tensor.matmul(
                    zps[ib][:], ones_sb[:],
                    e[ib][:, 2 * pr : 2 * pr + 2, :],
                    start=(pr == 0), stop=(pr == 3), perf_mode=DR,
                )

            def recip(ib):
                rz = rzpool.tile([P, IBS], F32, tag="rz")
                nc.vector.reciprocal_approx_fast(rz[:], zps[ib][:])
                return rz

            def o_block(ib, rz):
                isl = slice(ib * IBS, (ib + 1) * IBS)
                for ct in range(CT):
                    ops_ = opool.tile([P, IBS], F32, tag="o")
                    for pr in range(4):
                        nc.tensor.matmul(
                            ops_[:],
                            uT_sb[:, 2 * pr : 2 * pr + 2, ct * P : (ct + 1) * P],
                            e[ib][:, 2 * pr : 2 * pr + 2, :],
                            start=(pr == 0), stop=(pr == 3), perf_mode=DR,
                        )
                    om = ompool.tile([P, IBS], F32, tag="om")
                    nc.vector.tensor_tensor(om[:], ops_[:], rz[:], op=ALU.mult)
                    ot = outpool.tile([P, IBS], F32, tag="ot")
                    nc.vector.scalar_tensor_tensor(
                        ot[:], om[:], prm_sb[:, R_BPP, ct : ct + 1],
                        x_sb[:, ct, isl], op0=ALU.add, op1=ALU.add,
                    )
                    nc.sync.dma_start(outv[:, ct, isl], ot[:])

            for jt in range(JT):
                S_group(0, jt)
            for pr in range(3):
                Z_mm(0, pr)
            for jt in range(4):
                S_group(1, jt)
            Z_mm(0, 3)
            rz0 = recip(0)
            o_block(0, rz0)
            if b + 1 < BPC:
                stats_pre(xn_sb, stn, 0, f"x{b+1}")
            for jt in range(4, JT):
                S_group(1, jt)
            if b + 1 < BPC:
                gxn = stats_mm(stn, 0, f"x{b+1}")
            for pr in range(3):
                Z_mm(1, pr)
            if b + 1 < BPC:
                hn_sb = gn_apply(
                    xn_sb, hpool, "h", stats_post(gxn, R_GXS, R_GXB, f"x{b+1}")
                )
            Z_mm(1, 3)
            rz1 = recip(1)
            o_block(1, rz1)
            if b + 1 < BPC:
                stats_pre(ynx_sb, stn, 1, f"y{b+1}")
                pend = {"st": stn, "y": ynx_sb}
                h_sb, x_sb, y_sb = hn_sb, xn_sb, ynx_sb


_CACHE = {}


def _build(has_bq):
    key = ("nc", has_bq)
    if key in _CACHE:
        return _CACHE[key]
    nc = bacc.Bacc("TRN2", target_bir_lowering=False, debug=False)
    aps = {
        "xs": nc.dram_tensor("xs", [BPC, P, CT * HW], U16, kind="ExternalInput").ap(),
        "ys": nc.dram_tensor("ys", [BPC, P, CT * HW], U16, kind="ExternalInput").ap(),
        "A": nc.dram_tensor("A", [P, 4 * C], U8, kind="ExternalInput").ap(),
        "Bm": nc.dram_tensor("Bm", [P, 4 * C], U8, kind="ExternalInput").ap(),
        "prm": nc.dram_tensor("prm", [P, 5 * CT], F32, kind="ExternalInput").ap(),
        "amat": nc.dram_tensor("amat", [P, P], F32, kind="ExternalInput").ap(),
        "out": nc.dram_tensor("out", [BPC, P, CT * HW], F32, kind="ExternalOutput").ap(),
    }
    if has_bq:
        aps["gv"] = nc.dram_tensor("gv", [P, CT], U8, kind="ExternalInput").ap()
    with tile.TileContext(nc) as tc:
        _emit(tc, aps, has_bq)
    nc.compile()
    _CACHE[key] = nc
    return nc


def _pack_chw(a):
    """[*, C, HW] -> [*, P, CT*HW] matching SBUF layout c = t*128 + p."""
    lead = a.shape[:-2]
    a = a.reshape(*lead, CT, P, HW)
    a = np.moveaxis(a, -3, -2)          # [..., P, CT, HW]
    return np.ascontiguousarray(a.reshape(*lead, P, CT * HW))


def _unpack_chw(a):
    """[*, P, CT*HW] -> [*, C, HW]."""
    lead = a.shape[:-2]
    a = a.reshape(*lead, P, CT, HW)
    a = np.moveaxis(a, -2, -3)          # [..., CT, P, HW]
    return np.ascontiguousarray(a.reshape(*lead, CT * P, HW))


def _q8(a):
    return np.clip(a, -240.0, 240.0).astype(NPF8)


def _pack_w(wT, scale):
    """wT [cin, cout] -> fp8 bytes [P, 2*2*C]: [p, kpair, ktile2, cout],
    cin = (2*kpair + ktile2)*128 + p."""
    w8 = _q8(wT * scale).view(np.uint8)
    w8 = w8.reshape(2, 2, P, C).transpose(2, 0, 1, 3)
    return np.ascontiguousarray(w8.reshape(P, 4 * C))


def _host_inputs(x, y, norm_scale, norm_bias, norm1_scale, norm1_bias,
                 wq, bq, wk, bk, wv, bv, wp, bp):
    f = lambda a: np.ascontiguousarray(np.asarray(a, dtype=np.float32))
    x = f(x).reshape(B, C, HW)
    y = f(y).reshape(B, C, HW)
    wq, wk, wv, wp = f(wq), f(wk), f(wv), f(wp)
    A = wq.T @ wk                       # [cy, ch]
    Bm = wp @ wv                        # [co, ci]
    # bk cancels in softmax; bv folds into bp' because softmax rows sum to 1
    bpp = f(bp) + wp @ f(bv)
    prm = np.stack([f(norm_scale), f(norm_bias), f(norm1_scale), f(norm1_bias),
                    bpp]).astype(np.float32)
    prm = np.ascontiguousarray(
        prm.reshape(5, CT, P).transpose(2, 0, 1).reshape(P, 5 * CT)
    )
    amat = np.zeros((P, P), np.float32)
    for g in range(P // GSIZE):
        amat[g * GSIZE : (g + 1) * GSIZE, g * GSIZE : (g + 1) * GSIZE] = (
            1.0 / (GSIZE * HW)
        )
    has_bq = bool(np.any(np.asarray(bq)))
    shared = {
        "A": _pack_w(A.T, SA),          # lhsT[cin=ch, cout=cy]
        "Bm": _pack_w(Bm.T, SB),        # rhs[cin=ci, cout=co]
        "prm": prm, "amat": amat,
    }
    if has_bq:
        gv = wk.T @ f(bq)               # [ci]
        gv8 = _q8(gv).view(np.uint8).reshape(CT, P).T
        shared["gv"] = np.ascontiguousarray(gv8)

    xb = _pack_chw(x.astype(NPBF16).view(np.uint16))
    yb = _pack_chw(y.astype(NPBF16).view(np.uint16))
    in_maps = []
    for core in range(NCORES):
        sl = slice(core * BPC, (core + 1) * BPC)
        in_maps.append({"xs": xb[sl], "ys": yb[sl], **shared})
    return in_maps, has_bq


def _run(in_maps, has_bq, trace=False):
    nc = _build(has_bq)
    res = run_bass_kernel_spmd(
        nc, in_maps, core_ids=list(range(NCORES)), trace=trace
    )
    out = np.concatenate(
        [_unpack_chw(res.results[i]["out"]) for i in range(NCORES)], axis=0
    ).reshape(B, C, H, W)
    return out, res


def kernel(**inputs):
    in_maps, has_bq = _host_inputs(**inputs)
    out, _ = _run(in_maps, has_bq, trace=False)
    return out


# revision 11
# speedup vs baseline: 1.6485x; 1.2044x over previous
"""AttnBlock fusion kernel for Trainium2 (Bass/Tile), 8 NeuronCores.

Reference computation (per batch element b; c=512 channels, hw=1024 spatial):
    h  = GroupNorm(32, c)(x) ; k = Wk h + bk ; v = Wv h + bv
    y_ = GroupNorm(32, c)(y) ; q = Wq y_ + bq
    attn = softmax_j(q^T k / sqrt(c)) ; o = v @ attn^T ; out = x + Wp o + bp

Sharding: pure data parallel over batch (16 batches / 8 cores = 2 each).

Algebraic folds (host side, exact):
  * S = q^T k = y_^T (Wq^T Wk) h  -> A := Wq^T Wk precomputed; the q and k
    projections disappear (one matmul t = A h replaces both).
  * Wp (v @ P) = (Wp Wv) h @ P    -> Bm := Wp Wv precomputed; the v and
    proj_out projections disappear (u = Bm h replaces both).
  * bk adds a per-i constant to logits -> cancels in softmax.
  * bv contributes Wp bv exactly (softmax rows sum to 1) -> bp' = bp + Wp bv.
  * bq (zero in practice) handled by a compiled-in logit-bias path.

Precision scheme (validated vs reference: rel_l2 ~ 5.6e-3, gate 2e-2):
  * All five big matmuls run fp8(e4m3) with MatmulPerfMode.DoubleRow:
    2 contraction tiles per pass = 2x throughput over fp32r/bf16.
  * A, Bm scaled by 16 so t = A h and u = Bm h land in e4m3 range (+-240);
    1/16 is folded into the exp scale (t side) and into ones=16 for the
    Z row-sum matmul (u side, via 1/Z).
  * E = exp(s S - 3): the -3 shift is softmax-invariant and keeps
    max(E) ~ 31 < 240 so no fp8 overflow-to-inf.
  * x, y stored bf16 on chip (stats + gn + residual), accumulation fp32.

Engine split per batch (PE ~28us is the bound):
  PE      t/uT/S/Z/o DoubleRow matmuls + tiny GN aggregation matmul
  Scalar  exp(S), t PSUM->fp8 copies, GN sqrt
  DVE     bn_stats GN statistics, uT copies, 1/Z (fast approx), o*(1/Z)
  GpSimd  GN apply (bf16->fp8), residual add to output
"""

import math
import os
import sys
from contextlib import ExitStack

import numpy as np
import ml_dtypes

for _p in ("/opt/trn_rl_repo", "/root/.axon_site/_ro/trn_rl_repo"):
    if os.path.isdir(_p) and _p not in sys.path:
        sys.path.append(_p)

import concourse.bass as bass
import concourse.bacc as bacc
import concourse.mybir as mybir
import concourse.tile as tile
from concourse.bass_utils import run_bass_kernel_spmd

F32 = mybir.dt.float32
BF16 = mybir.dt.bfloat16
F8 = mybir.dt.float8e4
U8 = mybir.dt.uint8
U16 = mybir.dt.uint16
AF = mybir.ActivationFunctionType
ALU = mybir.AluOpType
DR = mybir.MatmulPerfMode.DoubleRow

B, C, H, W = 16, 512, 32, 32
HW = H * W                  # 1024
NCORES = 8
BPC = B // NCORES           # 2 batches per core
P = 128                     # SBUF partitions
CT = C // P                 # 4 channel tiles
JT = HW // P                # 8 key-position tiles
IBS = 512                   # query positions per i-block
IB = HW // IBS              # 2 i-blocks
GROUPS = 32
GSIZE = C // GROUPS         # 16 channels per group
EPS = 1e-6
SM = float(C) ** -0.5
SA = 16.0                   # scale folded into A
SB = 16.0                   # scale folded into Bm (and into ones for Z)
EXPS = SM / SA
EXPB = -3.0                 # softmax-invariant logit shift, keeps E < 240

NPF8 = ml_dtypes.float8_e4m3   # IEEE e4m3 (bias 7, max 240) == TRN FP8_EXP4
NPBF16 = ml_dtypes.bfloat16


def _emit(tc, aps, has_bq, has_bpp):
    nc = tc.nc
    xs, ys, out = aps["xs"], aps["ys"], aps["out"]

    with ExitStack() as ctx:
        cpool = ctx.enter_context(tc.tile_pool(name="const", bufs=1))
        wpool = ctx.enter_context(tc.tile_pool(name="w", bufs=1))
        xpool = ctx.enter_context(tc.tile_pool(name="xin", bufs=2))
        ypool = ctx.enter_context(tc.tile_pool(name="yin", bufs=2))
        hpool = ctx.enter_context(tc.tile_pool(name="hb", bufs=2))
        ynpool = ctx.enter_context(tc.tile_pool(name="ynb", bufs=2))
        tpool = ctx.enter_context(tc.tile_pool(name="tb", bufs=2))
        upool = ctx.enter_context(tc.tile_pool(name="ub", bufs=2))
        epool = ctx.enter_context(tc.tile_pool(name="eb", bufs=2))
        stpool = ctx.enter_context(tc.tile_pool(name="st", bufs=2))
        smpool = ctx.enter_context(tc.tile_pool(name="sm", bufs=3))
        ompool = ctx.enter_context(tc.tile_pool(name="om", bufs=3))
        outpool = ctx.enter_context(tc.tile_pool(name="outb", bufs=2))
        rzpool = ctx.enter_context(tc.tile_pool(name="rz", bufs=2))
        pspool = ctx.enter_context(tc.tile_pool(name="ps", bufs=3, space="PSUM"))
        zpool = ctx.enter_context(tc.tile_pool(name="z", bufs=1, space="PSUM"))
        opool = ctx.enter_context(tc.tile_pool(name="o", bufs=4, space="PSUM"))

        # ---- constants / weights ----
        prm_sb = cpool.tile([P, 5, CT], F32)
        nc.gpsimd.dma_start(prm_sb[:], aps["prm"].rearrange("p (q t) -> p q t", t=CT))
        amat_sb = cpool.tile([P, P], F32)
        nc.gpsimd.dma_start(amat_sb[:], aps["amat"][:])
        ones_sb = cpool.tile([P, 2, P], F8)
        nc.vector.memset(ones_sb[:], SB)
        expb_sb = cpool.tile([P, 1], F32)
        nc.vector.memset(expb_sb[:], EXPB)
        A_sb = wpool.tile([P, 2, 2, C], F8)
        nc.gpsimd.dma_start(
            A_sb[:], aps["A"].rearrange("p (a b o) -> p a b o", a=2, b=2).bitcast(F8)
        )
        Bm_sb = wpool.tile([P, 2, 2, C], F8)
        nc.gpsimd.dma_start(
            Bm_sb[:], aps["Bm"].rearrange("p (a b o) -> p a b o", a=2, b=2).bitcast(F8)
        )
        if has_bq:
            g_sb = cpool.tile([P, CT], F8)
            nc.gpsimd.dma_start(g_sb[:], aps["gv"].bitcast(F8))

        def load_xy(b):
            """Two half-tensor DMAs per input so stats can start early."""
            x_sb = xpool.tile([P, CT, HW], BF16, tag="x")
            v = xs[b].rearrange("p (t n) -> p t n", n=HW).bitcast(BF16)
            for hf in range(2):
                nc.sync.dma_start(x_sb[:, 2 * hf : 2 * hf + 2, :],
                                  v[:, 2 * hf : 2 * hf + 2, :])
            y_sb = ypool.tile([P, CT, HW], BF16, tag="y")
            v = ys[b].rearrange("p (t n) -> p t n", n=HW).bitcast(BF16)
            for hf in range(2):
                nc.sync.dma_start(y_sb[:, 2 * hf : 2 * hf + 2, :],
                                  v[:, 2 * hf : 2 * hf + 2, :])
            return x_sb, y_sb

        def stats_pre(src, st, u):
            """DVE bn_stats: per-channel [mean, var, mean^2] -> st[:, u]."""
            for t in range(CT):
                bns = smpool.tile([P, 2, 6], F32, tag="bns")
                for h2 in range(2):
                    nc.vector.bn_stats(
                        bns[:, h2, :], src[:, t, h2 * 512 : (h2 + 1) * 512]
                    )
                nc.vector.bn_aggr(st[:, u, t, 0:2], bns[:])
            nc.vector.tensor_tensor(
                st[:, u, :, 2], st[:, u, :, 0], st[:, u, :, 0], op=ALU.mult
            )

        def stats_mm(st):
            """PE: per-group averaging of [mean, var, mean^2] for x and y."""
            gt = pspool.tile([P, IBS], F32, tag="ps", name="gps")
            gps = gt[:, 0 : 2 * CT * 3]
            nc.tensor.matmul(gps, amat_sb[:], st[:], start=True, stop=True)
            return gps

        def stats_post(gps, uid):
            """a = rstd*gamma, mb = beta - mean*a for x and y: [P, 2, CT]."""
            g = smpool.tile([P, 2, CT, 3], F32, tag=f"g{uid}")
            nc.vector.tensor_copy(g[:], gps)
            # var_g = E[var] + E[mean^2] - E[mean]^2 (equal-count partitions)
            msq = smpool.tile([P, 2, CT], F32, tag=f"ms{uid}")
            nc.vector.tensor_tensor(msq[:], g[:, :, :, 0], g[:, :, :, 0], op=ALU.mult)
            var = smpool.tile([P, 2, CT], F32, tag=f"va{uid}")
            nc.vector.tensor_tensor(var[:], g[:, :, :, 1], g[:, :, :, 2], op=ALU.add)
            nc.vector.tensor_tensor(var[:], var[:], msq[:], op=ALU.subtract)
            nc.vector.tensor_scalar(var[:], var[:], EPS, None, op0=ALU.add)
            # rstd = 1/sqrt(var+eps), Newton-polished
            std = smpool.tile([P, 2, CT], F32, tag=f"sd{uid}")
            nc.scalar.activation(std[:], var[:], AF.Sqrt)
            r0 = smpool.tile([P, 2, CT], F32, tag=f"r0{uid}")
            nc.vector.reciprocal(r0[:], std[:])
            t7 = smpool.tile([P, 2, CT], F32, tag=f"t7{uid}")
            nc.vector.tensor_tensor(t7[:], r0[:], r0[:], op=ALU.mult)
            nc.vector.tensor_tensor(t7[:], var[:], t7[:], op=ALU.mult)
            nc.vector.tensor_scalar(t7[:], t7[:], -0.5, 1.5, op0=ALU.mult, op1=ALU.add)
            rstd = smpool.tile([P, 2, CT], F32, tag=f"rs{uid}")
            nc.vector.tensor_tensor(rstd[:], r0[:], t7[:], op=ALU.mult)
            a = smpool.tile([P, 2, CT], F32, tag=f"a{uid}")
            nc.vector.tensor_tensor(a[:], rstd[:], prm_sb[:, 0:2, :], op=ALU.mult)
            mb = smpool.tile([P, 2, CT], F32, tag=f"mb{uid}")
            nc.vector.tensor_tensor(mb[:], g[:, :, :, 0], a[:], op=ALU.mult)
            nc.vector.tensor_tensor(mb[:], prm_sb[:, 2:4, :], mb[:], op=ALU.subtract)
            return a, mb

        def gn_apply(src, pool, tag, ab, u, engine):
            a, mb = ab
            d = pool.tile([P, CT, HW], F8, tag=tag)
            for t in range(CT):
                engine.tensor_scalar(
                    d[:, t, :], src[:, t, :], a[:, u, t : t + 1],
                    mb[:, u, t : t + 1], op0=ALU.mult, op1=ALU.add,
                )
            return d

        # ---- prologue: batch 0 GN fully ----
        x_sb, y_sb = load_xy(0)
        st0 = stpool.tile([P, 2, CT, 3], F32, tag="st")
        stats_pre(x_sb, st0, 0)
        stats_pre(y_sb, st0, 1)
        ab = stats_post(stats_mm(st0), "b0")
        h_sb = gn_apply(x_sb, hpool, "h", ab, 0, nc.vector)
        yn_sb = gn_apply(y_sb, ynpool, "yn", ab, 1, nc.gpsimd)

        for b in range(BPC):
            outv = out[b].rearrange("p (t n) -> p t n", n=HW)

            # ---- P1a: t = A h  (t[cy, j], fp8; copies on Scalar) ----
            t_sb = tpool.tile([P, CT, HW], F8, tag="t")
            for nh in range(IB):
                for mt in range(CT):
                    ps = pspool.tile([P, IBS], F32, tag="ps")
                    for kp in range(2):
                        nc.tensor.matmul(
                            ps[:],
                            A_sb[:, kp, :, mt * P : (mt + 1) * P],
                            h_sb[:, 2 * kp : 2 * kp + 2, nh * IBS : (nh + 1) * IBS],
                            start=(kp == 0), stop=(kp == 1), perf_mode=DR,
                        )
                    nc.scalar.copy(t_sb[:, mt, nh * IBS : (nh + 1) * IBS], ps[:])

            if b + 1 < BPC:
                xn_sb, yn2_sb = load_xy(b + 1)
                stn = stpool.tile([P, 2, CT, 3], F32, tag="st")

            # ---- P1b: uT = h^T Bm^T  (uT[j, co], fp8; copies on DVE) ----
            uT_sb = upool.tile([P, JT, C], F8, tag="u")
            for jt in range(JT):
                ps = pspool.tile([P, C], F32, tag="ps")
                for kp in range(2):
                    nc.tensor.matmul(
                        ps[:],
                        h_sb[:, 2 * kp : 2 * kp + 2, jt * P : (jt + 1) * P],
                        Bm_sb[:, kp, :, :],
                        start=(kp == 0), stop=(kp == 1), perf_mode=DR,
                    )
                nc.vector.tensor_copy(uT_sb[:, jt, :], ps[:])

            # optional bq logit bias: r[j] = g^T h, bias = SM*r + EXPB
            if has_bq:
                rps = zpool.tile([P, JT], F32, tag="z", name="rb")
                for jt in range(JT):
                    for kt in range(CT):
                        nc.tensor.matmul(
                            rps[:, jt : jt + 1],
                            h_sb[:, kt, jt * P : (jt + 1) * P],
                            g_sb[:, kt : kt + 1],
                            start=(kt == 0), stop=(kt == CT - 1),
                        )
                bias_sb = smpool.tile([P, JT], F32, tag="bia")
                nc.vector.tensor_scalar(
                    bias_sb[:], rps[:], SM, EXPB, op0=ALU.mult, op1=ALU.add
                )

            # ---- attention ----
            e = [
                epool.tile([P, JT, IBS], F8, tag=f"e{ib}", name=f"e{ib}")
                for ib in range(IB)
            ]
            zps = {}

            def S_group(ib, jt):
                ps = pspool.tile([P, IBS], F32, tag="ps")
                for kp in range(2):
                    nc.tensor.matmul(
                        ps[:],
                        t_sb[:, 2 * kp : 2 * kp + 2, jt * P : (jt + 1) * P],
                        yn_sb[:, 2 * kp : 2 * kp + 2, ib * IBS : (ib + 1) * IBS],
                        start=(kp == 0), stop=(kp == 1), perf_mode=DR,
                    )
                bias = bias_sb[:, jt : jt + 1] if has_bq else expb_sb[:]
                nc.scalar.activation(
                    e[ib][:, jt, :], ps[:], AF.Exp, bias=bias, scale=EXPS
                )

            def Z_mm(ib, pr):
                if pr == 0:
                    zps[ib] = zpool.tile([P, IBS], F32, tag="z", name="z")
                nc.tensor.matmul(
                    zps[ib][:], ones_sb[:],
                    e[ib][:, 2 * pr : 2 * pr + 2, :],
                    start=(pr == 0), stop=(pr == 3), perf_mode=DR,
                )

            def recip(ib):
                rz = rzpool.tile([P, IBS], F32, tag="rz")
                nc.vector.reciprocal_approx_fast(rz[:], zps[ib][:])
                return rz

            def o_block(ib, rz, ot, xres):
                isl = slice(ib * IBS, (ib + 1) * IBS)
                for ct in range(CT):
                    ops_ = opool.tile([P, IBS], F32, tag="o")
                    for pr in range(4):
                        nc.tensor.matmul(
                            ops_[:],
                            uT_sb[:, 2 * pr : 2 * pr + 2, ct * P : (ct + 1) * P],
                            e[ib][:, 2 * pr : 2 * pr + 2, :],
                            start=(pr == 0), stop=(pr == 3), perf_mode=DR,
                        )
                    om = ompool.tile([P, IBS], F32, tag="om")
                    nc.vector.tensor_tensor(om[:], ops_[:], rz[:], op=ALU.mult)
                    nc.gpsimd.tensor_tensor(
                        ot[:, ct, :], om[:], xres[:, ct, isl], op=ALU.add
                    )
                nc.sync.dma_start(outv[:, :, isl], ot[:])

            # residual source (bpp folded in only when nonzero)
            if has_bpp:
                xres = outpool.tile([P, CT, HW], F32, tag="xb", name="xb")
                for t in range(CT):
                    nc.gpsimd.tensor_scalar(
                        xres[:, t, :], x_sb[:, t, :], prm_sb[:, 4, t : t + 1],
                        None, op0=ALU.add,
                    )
            else:
                xres = x_sb

            ot0 = outpool.tile([P, CT, IBS], F32, tag="ot")
            ot1 = outpool.tile([P, CT, IBS], F32, tag="ot")

            for jt in range(JT):
                S_group(0, jt)
            for pr in range(3):
                Z_mm(0, pr)
            for jt in range(4):
                S_group(1, jt)
            Z_mm(0, 3)
            rz0 = recip(0)
            o_block(0, rz0, ot0, xres)
            if b + 1 < BPC:
                stats_pre(xn_sb, stn, 0)
            for jt in range(4, JT):
                S_group(1, jt)
            if b + 1 < BPC:
                stats_pre(yn2_sb, stn, 1)
            for pr in range(3):
                Z_mm(1, pr)
            if b + 1 < BPC:
                gpsn = stats_mm(stn)
            Z_mm(1, 3)
            rz1 = recip(1)
            o_block(1, rz1, ot1, xres)
            if b + 1 < BPC:
                abn = stats_post(gpsn, f"b{b+1}")
                h_sb = gn_apply(xn_sb, hpool, "h", abn, 0, nc.gpsimd)
                yn_sb = gn_apply(yn2_sb, ynpool, "yn", abn, 1, nc.gpsimd)
                x_sb, y_sb = xn_sb, yn2_sb


_CACHE = {}


def _build(has_bq, has_bpp):
    key = ("nc", has_bq, has_bpp)
    if key in _CACHE:
        return _CACHE[key]
    nc = bacc.Bacc("TRN2", target_bir_lowering=False, debug=False)
    aps = {
        "xs": nc.dram_tensor("xs", [BPC, P, CT * HW], U16, kind="ExternalInput").ap(),
        "ys": nc.dram_tensor("ys", [BPC, P, CT * HW], U16, kind="ExternalInput").ap(),
        "A": nc.dram_tensor("A", [P, 4 * C], U8, kind="ExternalInput").ap(),
        "Bm": nc.dram_tensor("Bm", [P, 4 * C], U8, kind="ExternalInput").ap(),
        "prm": nc.dram_tensor("prm", [P, 5 * CT], F32, kind="ExternalInput").ap(),
        "amat": nc.dram_tensor("amat", [P, P], F32, kind="ExternalInput").ap(),
        "out": nc.dram_tensor("out", [BPC, P, CT * HW], F32, kind="ExternalOutput").ap(),
    }
    if has_bq:
        aps["gv"] = nc.dram_tensor("gv", [P, CT], U8, kind="ExternalInput").ap()
    with tile.TileContext(nc) as tc:
        _emit(tc, aps, has_bq, has_bpp)
    nc.compile()
    _CACHE[key] = nc
    return nc


def _pack_chw(a):
    """[*, C, HW] -> [*, P, CT*HW] matching SBUF layout c = t*128 + p."""
    lead = a.shape[:-2]
    a = a.reshape(*lead, CT, P, HW)
    a = np.moveaxis(a, -3, -2)          # [..., P, CT, HW]
    return np.ascontiguousarray(a.reshape(*lead, P, CT * HW))


def _unpack_chw(a):
    """[*, P, CT*HW] -> [*, C, HW]."""
    lead = a.shape[:-2]
    a = a.reshape(*lead, P, CT, HW)
    a = np.moveaxis(a, -2, -3)          # [..., CT, P, HW]
    return np.ascontiguousarray(a.reshape(*lead, CT * P, HW))


def _q8(a):
    return np.clip(a, -240.0, 240.0).astype(NPF8)


def _pack_w(wT, scale):
    """wT [cin, cout] -> fp8 bytes [P, 2*2*C]: [p, kpair, ktile2, cout],
    cin = (2*kpair + ktile2)*128 + p."""
    w8 = _q8(wT * scale).view(np.uint8)
    w8 = w8.reshape(2, 2, P, C).transpose(2, 0, 1, 3)
    return np.ascontiguousarray(w8.reshape(P, 4 * C))


def _host_inputs(x, y, norm_scale, norm_bias, norm1_scale, norm1_bias,
                 wq, bq, wk, bk, wv, bv, wp, bp):
    f = lambda a: np.ascontiguousarray(np.asarray(a, dtype=np.float32))
    x = f(x).reshape(B, C, HW)
    y = f(y).reshape(B, C, HW)
    wq, wk, wv, wp = f(wq), f(wk), f(wv), f(wp)
    A = wq.T @ wk                       # [cy, ch]
    Bm = wp @ wv                        # [co, ci]
    # bk cancels in softmax; bv folds into bp' because softmax rows sum to 1
    bpp = f(bp) + wp @ f(bv)
    # rows: [gamma_x, gamma_y, beta_x, beta_y, bpp]
    prm = np.stack([f(norm_scale), f(norm1_scale), f(norm_bias), f(norm1_bias),
                    bpp]).astype(np.float32)
    prm = np.ascontiguousarray(
        prm.reshape(5, CT, P).transpose(2, 0, 1).reshape(P, 5 * CT)
    )
    amat = np.zeros((P, P), np.float32)
    for g in range(P // GSIZE):
        amat[g * GSIZE : (g + 1) * GSIZE, g * GSIZE : (g + 1) * GSIZE] = 1.0 / GSIZE
    has_bq = bool(np.any(np.asarray(bq)))
    has_bpp = bool(np.any(bpp))
    shared = {
        "A": _pack_w(A.T, SA),          # lhsT[cin=ch, cout=cy]
        "Bm": _pack_w(Bm.T, SB),        # rhs[cin=ci, cout=co]
        "prm": prm, "amat": amat,
    }
    if has_bq:
        gv = wk.T @ f(bq)               # [ci]
        gv8 = _q8(gv).view(np.uint8).reshape(CT, P).T
        shared["gv"] = np.ascontiguousarray(gv8)

    xb = _pack_chw(x.astype(NPBF16).view(np.uint16))
    yb = _pack_chw(y.astype(NPBF16).view(np.uint16))
    in_maps = []
    for core in range(NCORES):
        sl = slice(core * BPC, (core + 1) * BPC)
        in_maps.append({"xs": xb[sl], "ys": yb[sl], **shared})
    return in_maps, has_bq, has_bpp


def _run(in_maps, has_bq, has_bpp, trace=False):
    nc = _build(has_bq, has_bpp)
    res = run_bass_kernel_spmd(
        nc, in_maps, core_ids=list(range(NCORES)), trace=trace
    )
    out = np.concatenate(
        [_unpack_chw(res.results[i]["out"]) for i in range(NCORES)], axis=0
    ).reshape(B, C, H, W)
    return out, res


def kernel(**inputs):
    in_maps, has_bq, has_bpp = _host_inputs(**inputs)
    out, _ = _run(in_maps, has_bq, has_bpp, trace=False)
    return out


# revision 14
# speedup vs baseline: 1.7838x; 1.0821x over previous
"""AttnBlock fusion kernel for Trainium2 (Bass/Tile), 8 NeuronCores.

Reference computation (per batch element b; c=512 channels, hw=1024 spatial):
    h  = GroupNorm(32, c)(x) ; k = Wk h + bk ; v = Wv h + bv
    y_ = GroupNorm(32, c)(y) ; q = Wq y_ + bq
    attn = softmax_j(q^T k / sqrt(c)) ; o = v @ attn^T ; out = x + Wp o + bp

Sharding: pure data parallel over batch (16 batches / 8 cores = 2 each).

Algebraic folds (host side, exact):
  * S = q^T k = y_^T (Wq^T Wk) h  -> A := Wq^T Wk precomputed; the q and k
    projections disappear (one matmul t = A h replaces both).
  * Wp (v @ P) = (Wp Wv) h @ P    -> Bm := Wp Wv precomputed; the v and
    proj_out projections disappear (u = Bm h replaces both).
  * bk adds a per-i constant to logits -> cancels in softmax.
  * bv contributes Wp bv exactly (softmax rows sum to 1) -> bp' = bp + Wp bv.
  * bq (zero in practice) handled by a compiled-in logit-bias path.

Precision scheme (validated vs reference: rel_l2 ~ 5.6e-3, gate 2e-2):
  * All five big matmuls run fp8(e4m3) with MatmulPerfMode.DoubleRow:
    2 contraction tiles per pass = 2x throughput over fp32r/bf16.
  * A, Bm scaled by 16 so t = A h and u = Bm h land in e4m3 range (+-240);
    1/16 is folded into the exp scale (t side) and into ones=16 for the
    Z row-sum matmul (u side, via 1/Z).
  * E = exp(s S - 3): the -3 shift is softmax-invariant and keeps
    max(E) ~ 31 < 240 so no fp8 overflow-to-inf.
  * x, y stored bf16 on chip (stats + gn + residual), accumulation fp32.

Engine split per batch (PE ~28us is the bound):
  PE      t/uT/S/Z/o DoubleRow matmuls + tiny GN aggregation matmul
  Scalar  exp(S), t PSUM->fp8 copies, GN sqrt
  DVE     bn_stats GN statistics, uT copies, 1/Z (fast approx), o*(1/Z)
  GpSimd  GN apply (bf16->fp8), residual add to output
"""

import math
import os
import sys
from contextlib import ExitStack

import numpy as np
import ml_dtypes

for _p in ("/opt/trn_rl_repo", "/root/.axon_site/_ro/trn_rl_repo"):
    if os.path.isdir(_p) and _p not in sys.path:
        sys.path.append(_p)

import concourse.bass as bass
import concourse.bacc as bacc
import concourse.mybir as mybir
import concourse.tile as tile
from concourse.bass_utils import run_bass_kernel_spmd

F32 = mybir.dt.float32
BF16 = mybir.dt.bfloat16
F8 = mybir.dt.float8e4
U8 = mybir.dt.uint8
U16 = mybir.dt.uint16
AF = mybir.ActivationFunctionType
ALU = mybir.AluOpType
DR = mybir.MatmulPerfMode.DoubleRow

B, C, H, W = 16, 512, 32, 32
HW = H * W                  # 1024
NCORES = 8
BPC = B // NCORES           # 2 batches per core
P = 128                     # SBUF partitions
CT = C // P                 # 4 channel tiles
JT = HW // P                # 8 key-position tiles
IBS = 512                   # query positions per i-block
IB = HW // IBS              # 2 i-blocks
GROUPS = 32
GSIZE = C // GROUPS         # 16 channels per group
EPS = 1e-6
SM = float(C) ** -0.5
SA = 16.0                   # scale folded into A
SB = 16.0                   # scale folded into Bm (and into ones for Z)
EXPS = SM / SA
EXPB = -3.0                 # softmax-invariant logit shift, keeps E < 240

NPF8 = ml_dtypes.float8_e4m3   # IEEE e4m3 (bias 7, max 240) == TRN FP8_EXP4
NPBF16 = ml_dtypes.bfloat16


def _emit(tc, aps, has_bq, has_bpp):
    nc = tc.nc
    xs, ys, out = aps["xs"], aps["ys"], aps["out"]

    with ExitStack() as ctx:
        cpool = ctx.enter_context(tc.tile_pool(name="const", bufs=1))
        wpool = ctx.enter_context(tc.tile_pool(name="w", bufs=1))
        xpool = ctx.enter_context(tc.tile_pool(name="xin", bufs=2))
        ypool = ctx.enter_context(tc.tile_pool(name="yin", bufs=2))
        hpool = ctx.enter_context(tc.tile_pool(name="hb", bufs=2))
        ynpool = ctx.enter_context(tc.tile_pool(name="ynb", bufs=2))
        tpool = ctx.enter_context(tc.tile_pool(name="tb", bufs=2))
        upool = ctx.enter_context(tc.tile_pool(name="ub", bufs=2))
        epool = ctx.enter_context(tc.tile_pool(name="eb", bufs=2))
        stpool = ctx.enter_context(tc.tile_pool(name="st", bufs=2))
        smpool = ctx.enter_context(tc.tile_pool(name="sm", bufs=3))
        ompool = ctx.enter_context(tc.tile_pool(name="om", bufs=3))
        outpool = ctx.enter_context(tc.tile_pool(name="outb", bufs=2))
        rzpool = ctx.enter_context(tc.tile_pool(name="rz", bufs=2))
        pspool = ctx.enter_context(tc.tile_pool(name="ps", bufs=3, space="PSUM"))
        zpool = ctx.enter_context(tc.tile_pool(name="z", bufs=1, space="PSUM"))
        opool = ctx.enter_context(tc.tile_pool(name="o", bufs=4, space="PSUM"))

        # ---- constants / weights ----
        prm_sb = cpool.tile([P, 5, CT], F32)
        nc.gpsimd.dma_start(prm_sb[:], aps["prm"].rearrange("p (q t) -> p q t", t=CT))
        amat_sb = cpool.tile([P, P], F32)
        nc.gpsimd.dma_start(amat_sb[:], aps["amat"][:])
        ones_sb = cpool.tile([P, 2, P], F8)
        nc.vector.memset(ones_sb[:], SB)
        expb_sb = cpool.tile([P, 1], F32)
        nc.vector.memset(expb_sb[:], EXPB)
        A_sb = wpool.tile([P, 2, 2, C], F8)
        nc.gpsimd.dma_start(
            A_sb[:], aps["A"].rearrange("p (a b o) -> p a b o", a=2, b=2).bitcast(F8)
        )
        Bm_sb = wpool.tile([P, 2, 2, C], F8)
        nc.gpsimd.dma_start(
            Bm_sb[:], aps["Bm"].rearrange("p (a b o) -> p a b o", a=2, b=2).bitcast(F8)
        )
        if has_bq:
            g_sb = cpool.tile([P, CT], F8)
            nc.gpsimd.dma_start(g_sb[:], aps["gv"].bitcast(F8))

        def load_xy(b):
            """Two half-tensor DMAs per input so stats can start early."""
            x_sb = xpool.tile([P, CT, HW], BF16, tag="x")
            v = xs[b].rearrange("p (t n) -> p t n", n=HW).bitcast(BF16)
            for hf in range(2):
                nc.sync.dma_start(x_sb[:, 2 * hf : 2 * hf + 2, :],
                                  v[:, 2 * hf : 2 * hf + 2, :])
            y_sb = ypool.tile([P, CT, HW], BF16, tag="y")
            v = ys[b].rearrange("p (t n) -> p t n", n=HW).bitcast(BF16)
            for hf in range(2):
                nc.sync.dma_start(y_sb[:, 2 * hf : 2 * hf + 2, :],
                                  v[:, 2 * hf : 2 * hf + 2, :])
            return x_sb, y_sb

        def stats_pre(src, st, u):
            """DVE bn_stats: per-channel [mean, var, mean^2] -> st[:, u]."""
            for t in range(CT):
                bns = smpool.tile([P, 2, 6], F32, tag="bns")
                for h2 in range(2):
                    nc.vector.bn_stats(
                        bns[:, h2, :], src[:, t, h2 * 512 : (h2 + 1) * 512]
                    )
                nc.vector.bn_aggr(st[:, u, t, 0:2], bns[:])
            nc.vector.tensor_tensor(
                st[:, u, :, 2], st[:, u, :, 0], st[:, u, :, 0], op=ALU.mult
            )

        def stats_mm(st, u):
            """PE: per-group averaging of [mean, var, mean^2] for one tensor."""
            gt = pspool.tile([P, IBS], F32, tag="ps", name="gps")
            gps = gt[:, 0 : CT * 3]
            nc.tensor.matmul(gps, amat_sb[:], st[:, u], start=True, stop=True)
            return gps

        def stats_post(gps, u, uid):
            """a = rstd*gamma, mb = beta - mean*a for one tensor: [P, CT]."""
            g = smpool.tile([P, CT, 3], F32, tag=f"g{uid}")
            nc.vector.tensor_copy(g[:], gps)
            # var_g = E[var] + E[mean^2] - E[mean]^2 (equal-count partitions)
            msq = smpool.tile([P, CT], F32, tag=f"ms{uid}")
            nc.vector.tensor_tensor(msq[:], g[:, :, 0], g[:, :, 0], op=ALU.mult)
            var = smpool.tile([P, CT], F32, tag=f"va{uid}")
            nc.vector.tensor_tensor(var[:], g[:, :, 1], g[:, :, 2], op=ALU.add)
            nc.vector.tensor_tensor(var[:], var[:], msq[:], op=ALU.subtract)
            nc.vector.tensor_scalar(var[:], var[:], EPS, None, op0=ALU.add)
            # rstd = 1/sqrt(var+eps), Newton-polished
            std = smpool.tile([P, CT], F32, tag=f"sd{uid}")
            nc.scalar.activation(std[:], var[:], AF.Sqrt)
            r0 = smpool.tile([P, CT], F32, tag=f"r0{uid}")
            nc.vector.reciprocal(r0[:], std[:])
            t7 = smpool.tile([P, CT], F32, tag=f"t7{uid}")
            nc.vector.tensor_tensor(t7[:], r0[:], r0[:], op=ALU.mult)
            nc.vector.tensor_tensor(t7[:], var[:], t7[:], op=ALU.mult)
            nc.vector.tensor_scalar(t7[:], t7[:], -0.5, 1.5, op0=ALU.mult, op1=ALU.add)
            rstd = smpool.tile([P, CT], F32, tag=f"rs{uid}")
            nc.vector.tensor_tensor(rstd[:], r0[:], t7[:], op=ALU.mult)
            a = smpool.tile([P, CT], F32, tag=f"a{uid}")
            nc.vector.tensor_tensor(a[:], rstd[:], prm_sb[:, u, :], op=ALU.mult)
            mb = smpool.tile([P, CT], F32, tag=f"mb{uid}")
            nc.vector.tensor_tensor(mb[:], g[:, :, 0], a[:], op=ALU.mult)
            nc.vector.tensor_tensor(mb[:], prm_sb[:, 2 + u, :], mb[:], op=ALU.subtract)
            return a, mb

        def gn_apply(src, pool, tag, ab, engines):
            """Per-tile affine-normalize, tile t on engines[t] (bf16 -> fp8)."""
            a, mb = ab
            d = pool.tile([P, CT, HW], F8, tag=tag)
            for t in range(CT):
                eng = engines[t]
                if eng is nc.scalar:
                    nc.scalar.activation(
                        d[:, t, :], src[:, t, :], AF.Identity,
                        bias=mb[:, t : t + 1], scale=a[:, t : t + 1],
                    )
                else:
                    eng.tensor_scalar(
                        d[:, t, :], src[:, t, :], a[:, t : t + 1],
                        mb[:, t : t + 1], op0=ALU.mult, op1=ALU.add,
                    )
            return d

        V, G, SC = nc.vector, nc.gpsimd, nc.scalar

        # ---- prologue: batch 0 GN, x-chain first so t matmuls start early ----
        x_sb, y_sb = load_xy(0)
        st0 = stpool.tile([P, 2, CT, 3], F32, tag="st")
        stats_pre(x_sb, st0, 0)
        abx = stats_post(stats_mm(st0, 0), 0, "x0")
        h_sb = gn_apply(x_sb, hpool, "h", abx, [V, V, G, G])
        stats_pre(y_sb, st0, 1)
        aby = stats_post(stats_mm(st0, 1), 1, "y0")
        yn_sb = gn_apply(y_sb, ynpool, "yn", aby, [SC, SC, G, G])

        for b in range(BPC):
            outv = out[b].rearrange("p (t n) -> p t n", n=HW)

            # ---- P1a: t = A h  (t[cy, j], fp8; copies on Scalar) ----
            t_sb = tpool.tile([P, CT, HW], F8, tag="t")
            for nh in range(IB):
                for mt in range(CT):
                    ps = pspool.tile([P, IBS], F32, tag="ps")
                    for kp in range(2):
                        nc.tensor.matmul(
                            ps[:],
                            A_sb[:, kp, :, mt * P : (mt + 1) * P],
                            h_sb[:, 2 * kp : 2 * kp + 2, nh * IBS : (nh + 1) * IBS],
                            start=(kp == 0), stop=(kp == 1), perf_mode=DR,
                        )
                    nc.scalar.copy(t_sb[:, mt, nh * IBS : (nh + 1) * IBS], ps[:])

            if b + 1 < BPC:
                xn_sb, yn2_sb = load_xy(b + 1)
                stn = stpool.tile([P, 2, CT, 3], F32, tag="st")

            # ---- P1b: uT = h^T Bm^T  (uT[j, co], fp8; copies split Sc/DVE) ----
            uT_sb = upool.tile([P, JT, C], F8, tag="u")
            for jt in range(JT):
                ps = pspool.tile([P, C], F32, tag="ps")
                for kp in range(2):
                    nc.tensor.matmul(
                        ps[:],
                        h_sb[:, 2 * kp : 2 * kp + 2, jt * P : (jt + 1) * P],
                        Bm_sb[:, kp, :, :],
                        start=(kp == 0), stop=(kp == 1), perf_mode=DR,
                    )
                if jt < 4:
                    nc.scalar.copy(uT_sb[:, jt, :], ps[:])
                else:
                    nc.vector.tensor_copy(uT_sb[:, jt, :], ps[:])

            # optional bq logit bias: r[j] = g^T h, bias = SM*r + EXPB
            if has_bq:
                rps = zpool.tile([P, JT], F32, tag="z", name="rb")
                for jt in range(JT):
                    for kt in range(CT):
                        nc.tensor.matmul(
                            rps[:, jt : jt + 1],
                            h_sb[:, kt, jt * P : (jt + 1) * P],
                            g_sb[:, kt : kt + 1],
                            start=(kt == 0), stop=(kt == CT - 1),
                        )
                bias_sb = smpool.tile([P, JT], F32, tag="bia")
                nc.vector.tensor_scalar(
                    bias_sb[:], rps[:], SM, EXPB, op0=ALU.mult, op1=ALU.add
                )

            # ---- attention ----
            e = [
                epool.tile([P, JT, IBS], F8, tag=f"e{ib}", name=f"e{ib}")
                for ib in range(IB)
            ]
            zps = {}

            def S_group(ib, jt):
                ps = pspool.tile([P, IBS], F32, tag="ps")
                for kp in range(2):
                    nc.tensor.matmul(
                        ps[:],
                        t_sb[:, 2 * kp : 2 * kp + 2, jt * P : (jt + 1) * P],
                        yn_sb[:, 2 * kp : 2 * kp + 2, ib * IBS : (ib + 1) * IBS],
                        start=(kp == 0), stop=(kp == 1), perf_mode=DR,
                    )
                bias = bias_sb[:, jt : jt + 1] if has_bq else expb_sb[:]
                nc.scalar.activation(
                    e[ib][:, jt, :], ps[:], AF.Exp, bias=bias, scale=EXPS
                )

            def Z_mm(ib, pr):
                if pr == 0:
                    zps[ib] = zpool.tile([P, IBS], F32, tag="z", name="z")
                nc.tensor.matmul(
                    zps[ib][:], ones_sb[:],
                    e[ib][:, 2 * pr : 2 * pr + 2, :],
                    start=(pr == 0), stop=(pr == 3), perf_mode=DR,
                )

            def recip(ib):
                rz = rzpool.tile([P, IBS], F32, tag="rz")
                nc.vector.reciprocal_approx_fast(rz[:], zps[ib][:])
                return rz

            def o_block(ib, rz, ot, xres):
                isl = slice(ib * IBS, (ib + 1) * IBS)
                for ct in range(CT):
                    ops_ = opool.tile([P, IBS], F32, tag="o")
                    for pr in range(4):
                        nc.tensor.matmul(
                            ops_[:],
                            uT_sb[:, 2 * pr : 2 * pr + 2, ct * P : (ct + 1) * P],
                            e[ib][:, 2 * pr : 2 * pr + 2, :],
                            start=(pr == 0), stop=(pr == 3), perf_mode=DR,
                        )
                    om = ompool.tile([P, IBS], F32, tag="om")
                    nc.vector.tensor_tensor(om[:], ops_[:], rz[:], op=ALU.mult)
                    nc.gpsimd.tensor_tensor(
                        ot[:, ct, :], om[:], xres[:, ct, isl], op=ALU.add
                    )
                nc.sync.dma_start(outv[:, :, isl], ot[:])

            # residual source (bpp folded in only when nonzero)
            if has_bpp:
                xres = outpool.tile([P, CT, HW], F32, tag="xb", name="xb")
                for t in range(CT):
                    nc.gpsimd.tensor_scalar(
                        xres[:, t, :], x_sb[:, t, :], prm_sb[:, 4, t : t + 1],
                        None, op0=ALU.add,
                    )
            else:
                xres = x_sb

            ot0 = outpool.tile([P, CT, IBS], F32, tag="ot")
            ot1 = outpool.tile([P, CT, IBS], F32, tag="ot")

            for jt in range(JT):
                S_group(0, jt)
            for pr in range(3):
                Z_mm(0, pr)
            for jt in range(4):
                S_group(1, jt)
            Z_mm(0, 3)
            rz0 = recip(0)
            o_block(0, rz0, ot0, xres)
            if b + 1 < BPC:
                stats_pre(xn_sb, stn, 0)
            for jt in range(4, JT):
                S_group(1, jt)
            if b + 1 < BPC:
                stats_pre(yn2_sb, stn, 1)
            for pr in range(3):
                Z_mm(1, pr)
            if b + 1 < BPC:
                gxn = stats_mm(stn, 0)
                gyn = stats_mm(stn, 1)
            Z_mm(1, 3)
            rz1 = recip(1)
            if b + 1 < BPC:
                # finish next batch's GN before o_block(1) so its t matmuls
                # can start the moment this batch's PE work drains
                abxn = stats_post(gxn, 0, f"x{b+1}")
                hn_sb = gn_apply(xn_sb, hpool, "h", abxn, [V, V, G, G])
                abyn = stats_post(gyn, 1, f"y{b+1}")
                ynn_sb = gn_apply(yn2_sb, ynpool, "yn", abyn, [SC, SC, G, G])
            o_block(1, rz1, ot1, xres)
            if b + 1 < BPC:
                h_sb, yn_sb = hn_sb, ynn_sb
                x_sb, y_sb = xn_sb, yn2_sb


_CACHE = {}


def _build(has_bq, has_bpp):
    key = ("nc", has_bq, has_bpp)
    if key in _CACHE:
        return _CACHE[key]
    nc = bacc.Bacc("TRN2", target_bir_lowering=False, debug=False)
    aps = {
        "xs": nc.dram_tensor("xs", [BPC, P, CT * HW], U16, kind="ExternalInput").ap(),
        "ys": nc.dram_tensor("ys", [BPC, P, CT * HW], U16, kind="ExternalInput").ap(),
        "A": nc.dram_tensor("A", [P, 4 * C], U8, kind="ExternalInput").ap(),
        "Bm": nc.dram_tensor("Bm", [P, 4 * C], U8, kind="ExternalInput").ap(),
        "prm": nc.dram_tensor("prm", [P, 5 * CT], F32, kind="ExternalInput").ap(),
        "amat": nc.dram_tensor("amat", [P, P], F32, kind="ExternalInput").ap(),
        "out": nc.dram_tensor("out", [BPC, P, CT * HW], F32, kind="ExternalOutput").ap(),
    }
    if has_bq:
        aps["gv"] = nc.dram_tensor("gv", [P, CT], U8, kind="ExternalInput").ap()
    with tile.TileContext(nc) as tc:
        _emit(tc, aps, has_bq, has_bpp)
    nc.compile()
    _CACHE[key] = nc
    return nc


def _pack_chw(a):
    """[*, C, HW] -> [*, P, CT*HW] matching SBUF layout c = t*128 + p."""
    lead = a.shape[:-2]
    a = a.reshape(*lead, CT, P, HW)
    a = np.moveaxis(a, -3, -2)          # [..., P, CT, HW]
    return np.ascontiguousarray(a.reshape(*lead, P, CT * HW))


def _unpack_chw(a):
    """[*, P, CT*HW] -> [*, C, HW]."""
    lead = a.shape[:-2]
    a = a.reshape(*lead, P, CT, HW)
    a = np.moveaxis(a, -2, -3)          # [..., CT, P, HW]
    return np.ascontiguousarray(a.reshape(*lead, CT * P, HW))


def _q8(a):
    return np.clip(a, -240.0, 240.0).astype(NPF8)


def _pack_w(wT, scale):
    """wT [cin, cout] -> fp8 bytes [P, 2*2*C]: [p, kpair, ktile2, cout],
    cin = (2*kpair + ktile2)*128 + p."""
    w8 = _q8(wT * scale).view(np.uint8)
    w8 = w8.reshape(2, 2, P, C).transpose(2, 0, 1, 3)
    return np.ascontiguousarray(w8.reshape(P, 4 * C))


def _host_inputs(x, y, norm_scale, norm_bias, norm1_scale, norm1_bias,
                 wq, bq, wk, bk, wv, bv, wp, bp):
    f = lambda a: np.ascontiguousarray(np.asarray(a, dtype=np.float32))
    x = f(x).reshape(B, C, HW)
    y = f(y).reshape(B, C, HW)
    wq, wk, wv, wp = f(wq), f(wk), f(wv), f(wp)
    A = wq.T @ wk                       # [cy, ch]
    Bm = wp @ wv                        # [co, ci]
    # bk cancels in softmax; bv folds into bp' because softmax rows sum to 1
    bpp = f(bp) + wp @ f(bv)
    # rows: [gamma_x, gamma_y, beta_x, beta_y, bpp]
    prm = np.stack([f(norm_scale), f(norm1_scale), f(norm_bias), f(norm1_bias),
                    bpp]).astype(np.float32)
    prm = np.ascontiguousarray(
        prm.reshape(5, CT, P).transpose(2, 0, 1).reshape(P, 5 * CT)
    )
    amat = np.zeros((P, P), np.float32)
    for g in range(P // GSIZE):
        amat[g * GSIZE : (g + 1) * GSIZE, g * GSIZE : (g + 1) * GSIZE] = 1.0 / GSIZE
    has_bq = bool(np.any(np.asarray(bq)))
    has_bpp = bool(np.any(bpp))
    shared = {
        "A": _pack_w(A.T, SA),          # lhsT[cin=ch, cout=cy]
        "Bm": _pack_w(Bm.T, SB),        # rhs[cin=ci, cout=co]
        "prm": prm, "amat": amat,
    }
    if has_bq:
        gv = wk.T @ f(bq)               # [ci]
        gv8 = _q8(gv).view(np.uint8).reshape(CT, P).T
        shared["gv"] = np.ascontiguousarray(gv8)

    xb = _pack_chw(x.astype(NPBF16).view(np.uint16))
    yb = _pack_chw(y.astype(NPBF16).view(np.uint16))
    in_maps = []
    for core in range(NCORES):
        sl = slice(core * BPC, (core + 1) * BPC)
        in_maps.append({"xs": xb[sl], "ys": yb[sl], **shared})
    return in_maps, has_bq, has_bpp


def _run(in_maps, has_bq, has_bpp, trace=False):
    nc = _build(has_bq, has_bpp)
    res = run_bass_kernel_spmd(
        nc, in_maps, core_ids=list(range(NCORES)), trace=trace
    )
    out = np.concatenate(
        [_unpack_chw(res.results[i]["out"]) for i in range(NCORES)], axis=0
    ).reshape(B, C, H, W)
    return out, res


def kernel(**inputs):
    in_maps, has_bq, has_bpp = _host_inputs(**inputs)
    out, _ = _run(in_maps, has_bq, has_bpp, trace=False)
    return out


# revision 15
# speedup vs baseline: 1.8764x; 1.0519x over previous
"""AttnBlock fusion kernel for Trainium2 (Bass/Tile), 8 NeuronCores.

Reference computation (per batch element b; c=512 channels, hw=1024 spatial):
    h  = GroupNorm(32, c)(x) ; k = Wk h + bk ; v = Wv h + bv
    y_ = GroupNorm(32, c)(y) ; q = Wq y_ + bq
    attn = softmax_j(q^T k / sqrt(c)) ; o = v @ attn^T ; out = x + Wp o + bp

Sharding: pure data parallel over batch (16 batches / 8 cores = 2 each).

Algebraic folds (host side, exact):
  * S = q^T k = y_^T (Wq^T Wk) h  -> A := Wq^T Wk precomputed; the q and k
    projections disappear (one matmul t = A h replaces both).
  * Wp (v @ P) = (Wp Wv) h @ P    -> Bm := Wp Wv precomputed; the v and
    proj_out projections disappear (u = Bm h replaces both).
  * bk adds a per-i constant to logits -> cancels in softmax.
  * bv contributes Wp bv exactly (softmax rows sum to 1) -> bp' = bp + Wp bv.
  * bq (zero in practice) handled by a compiled-in logit-bias path.

Precision scheme (validated vs reference: rel_l2 ~ 5.6e-3, gate 2e-2):
  * All five big matmuls run fp8(e4m3) with MatmulPerfMode.DoubleRow:
    2 contraction tiles per pass = 2x throughput over fp32r/bf16.
  * A, Bm scaled by 16 so t = A h and u = Bm h land in e4m3 range (+-240);
    1/16 is folded into the exp scale (t side) and into ones=16 for the
    Z row-sum matmul (u side, via 1/Z).
  * E = exp(s S - 3): the -3 shift is softmax-invariant and keeps
    max(E) ~ 31 < 240 so no fp8 overflow-to-inf.
  * x, y stored bf16 on chip (stats + gn + residual), accumulation fp32.

Engine split per batch (PE ~28us is the bound):
  PE      t/uT/S/Z/o DoubleRow matmuls + tiny GN aggregation matmul
  Scalar  exp(S), t PSUM->fp8 copies, GN sqrt
  DVE     bn_stats GN statistics, uT copies, 1/Z (fast approx), o*(1/Z)
  GpSimd  GN apply (bf16->fp8), residual add to output
"""

import math
import os
import sys
from contextlib import ExitStack

import numpy as np
import ml_dtypes

for _p in ("/opt/trn_rl_repo", "/root/.axon_site/_ro/trn_rl_repo"):
    if os.path.isdir(_p) and _p not in sys.path:
        sys.path.append(_p)

import concourse.bass as bass
import concourse.bacc as bacc
import concourse.mybir as mybir
import concourse.tile as tile
from concourse.bass_utils import run_bass_kernel_spmd

F32 = mybir.dt.float32
BF16 = mybir.dt.bfloat16
F8 = mybir.dt.float8e4
U8 = mybir.dt.uint8
U16 = mybir.dt.uint16
AF = mybir.ActivationFunctionType
ALU = mybir.AluOpType
DR = mybir.MatmulPerfMode.DoubleRow

B, C, H, W = 16, 512, 32, 32
HW = H * W                  # 1024
NCORES = 8
BPC = B // NCORES           # 2 batches per core
P = 128                     # SBUF partitions
CT = C // P                 # 4 channel tiles
JT = HW // P                # 8 key-position tiles
IBS = 512                   # query positions per i-block
IB = HW // IBS              # 2 i-blocks
GROUPS = 32
GSIZE = C // GROUPS         # 16 channels per group
EPS = 1e-6
SM = float(C) ** -0.5
SA = 16.0                   # scale folded into A
SB = 16.0                   # scale folded into Bm (and into ones for Z)
EXPS = SM / SA
EXPB = -3.0                 # softmax-invariant logit shift, keeps E < 240

NPF8 = ml_dtypes.float8_e4m3   # IEEE e4m3 (bias 7, max 240) == TRN FP8_EXP4
NPBF16 = ml_dtypes.bfloat16


def _emit(tc, aps, has_bq, has_bpp):
    nc = tc.nc
    xs, ys, out = aps["xs"], aps["ys"], aps["out"]

    with ExitStack() as ctx:
        cpool = ctx.enter_context(tc.tile_pool(name="const", bufs=1))
        wpool = ctx.enter_context(tc.tile_pool(name="w", bufs=1))
        xpool = ctx.enter_context(tc.tile_pool(name="xin", bufs=2))
        ypool = ctx.enter_context(tc.tile_pool(name="yin", bufs=2))
        hpool = ctx.enter_context(tc.tile_pool(name="hb", bufs=2))
        ynpool = ctx.enter_context(tc.tile_pool(name="ynb", bufs=2))
        tpool = ctx.enter_context(tc.tile_pool(name="tb", bufs=2))
        upool = ctx.enter_context(tc.tile_pool(name="ub", bufs=2))
        epool = ctx.enter_context(tc.tile_pool(name="eb", bufs=2))
        stpool = ctx.enter_context(tc.tile_pool(name="st", bufs=2))
        smpool = ctx.enter_context(tc.tile_pool(name="sm", bufs=3))
        ompool = ctx.enter_context(tc.tile_pool(name="om", bufs=3))
        outpool = ctx.enter_context(tc.tile_pool(name="outb", bufs=2))
        rzpool = ctx.enter_context(tc.tile_pool(name="rz", bufs=2))
        pspool = ctx.enter_context(tc.tile_pool(name="ps", bufs=3, space="PSUM"))
        zpool = ctx.enter_context(tc.tile_pool(name="z", bufs=1, space="PSUM"))
        opool = ctx.enter_context(tc.tile_pool(name="o", bufs=4, space="PSUM"))

        # ---- constants / weights ----
        prm_sb = cpool.tile([P, 5, CT], F32)
        nc.gpsimd.dma_start(prm_sb[:], aps["prm"].rearrange("p (q t) -> p q t", t=CT))
        amat_sb = cpool.tile([P, P], F32)
        nc.gpsimd.dma_start(amat_sb[:], aps["amat"][:])
        ones_sb = cpool.tile([P, 2, P], F8)
        nc.vector.memset(ones_sb[:], SB)
        expb_sb = cpool.tile([P, 1], F32)
        nc.vector.memset(expb_sb[:], EXPB)
        A_sb = wpool.tile([P, 2, 2, C], F8)
        nc.gpsimd.dma_start(
            A_sb[:], aps["A"].rearrange("p (a b o) -> p a b o", a=2, b=2).bitcast(F8)
        )
        Bm_sb = wpool.tile([P, 2, 2, C], F8)
        nc.gpsimd.dma_start(
            Bm_sb[:], aps["Bm"].rearrange("p (a b o) -> p a b o", a=2, b=2).bitcast(F8)
        )
        if has_bq:
            g_sb = cpool.tile([P, CT], F8)
            nc.gpsimd.dma_start(g_sb[:], aps["gv"].bitcast(F8))

        def load_xy(b):
            """Two half-tensor DMAs per input so stats can start early."""
            x_sb = xpool.tile([P, CT, HW], BF16, tag="x")
            v = xs[b].rearrange("p (t n) -> p t n", n=HW).bitcast(BF16)
            for hf in range(2):
                nc.sync.dma_start(x_sb[:, 2 * hf : 2 * hf + 2, :],
                                  v[:, 2 * hf : 2 * hf + 2, :])
            y_sb = ypool.tile([P, CT, HW], BF16, tag="y")
            v = ys[b].rearrange("p (t n) -> p t n", n=HW).bitcast(BF16)
            for hf in range(2):
                nc.sync.dma_start(y_sb[:, 2 * hf : 2 * hf + 2, :],
                                  v[:, 2 * hf : 2 * hf + 2, :])
            return x_sb, y_sb

        def stats_pre(src, st, u):
            """DVE bn_stats: per-channel [mean, var, mean^2] -> st[:, u]."""
            for t in range(CT):
                bns = smpool.tile([P, 2, 6], F32, tag="bns")
                for h2 in range(2):
                    nc.vector.bn_stats(
                        bns[:, h2, :], src[:, t, h2 * 512 : (h2 + 1) * 512]
                    )
                nc.vector.bn_aggr(st[:, u, t, 0:2], bns[:])
            nc.vector.tensor_tensor(
                st[:, u, :, 2], st[:, u, :, 0], st[:, u, :, 0], op=ALU.mult
            )

        def stats_mm(st, u):
            """PE: per-group averaging of [mean, var, mean^2] for one tensor."""
            gt = pspool.tile([P, IBS], F32, tag="ps", name="gps")
            gps = gt[:, 0 : CT * 3]
            nc.tensor.matmul(gps, amat_sb[:], st[:, u], start=True, stop=True)
            return gps

        def stats_post(gps, u, uid):
            """a = rstd*gamma, mb = beta - mean*a for one tensor: [P, CT]."""
            g = smpool.tile([P, CT, 3], F32, tag=f"g{uid}")
            nc.vector.tensor_copy(g[:], gps)
            # var_g = E[var] + E[mean^2] - E[mean]^2 (equal-count partitions)
            msq = smpool.tile([P, CT], F32, tag=f"ms{uid}")
            nc.vector.tensor_tensor(msq[:], g[:, :, 0], g[:, :, 0], op=ALU.mult)
            var = smpool.tile([P, CT], F32, tag=f"va{uid}")
            nc.vector.tensor_tensor(var[:], g[:, :, 1], g[:, :, 2], op=ALU.add)
            nc.vector.tensor_tensor(var[:], var[:], msq[:], op=ALU.subtract)
            nc.vector.tensor_scalar(var[:], var[:], EPS, None, op0=ALU.add)
            # rstd = 1/sqrt(var+eps), Newton-polished
            std = smpool.tile([P, CT], F32, tag=f"sd{uid}")
            nc.scalar.activation(std[:], var[:], AF.Sqrt)
            r0 = smpool.tile([P, CT], F32, tag=f"r0{uid}")
            nc.vector.reciprocal(r0[:], std[:])
            t7 = smpool.tile([P, CT], F32, tag=f"t7{uid}")
            nc.vector.tensor_tensor(t7[:], r0[:], r0[:], op=ALU.mult)
            nc.vector.tensor_tensor(t7[:], var[:], t7[:], op=ALU.mult)
            nc.vector.tensor_scalar(t7[:], t7[:], -0.5, 1.5, op0=ALU.mult, op1=ALU.add)
            rstd = smpool.tile([P, CT], F32, tag=f"rs{uid}")
            nc.vector.tensor_tensor(rstd[:], r0[:], t7[:], op=ALU.mult)
            a = smpool.tile([P, CT], F32, tag=f"a{uid}")
            nc.vector.tensor_tensor(a[:], rstd[:], prm_sb[:, u, :], op=ALU.mult)
            mb = smpool.tile([P, CT], F32, tag=f"mb{uid}")
            nc.vector.tensor_tensor(mb[:], g[:, :, 0], a[:], op=ALU.mult)
            nc.vector.tensor_tensor(mb[:], prm_sb[:, 2 + u, :], mb[:], op=ALU.subtract)
            return a, mb

        def gn_apply(src, pool, tag, ab, engines):
            """Per-tile affine-normalize, tile t on engines[t] (bf16 -> fp8)."""
            a, mb = ab
            d = pool.tile([P, CT, HW], F8, tag=tag)
            for t in range(CT):
                eng = engines[t]
                if eng is nc.scalar:
                    nc.scalar.activation(
                        d[:, t, :], src[:, t, :], AF.Identity,
                        bias=mb[:, t : t + 1], scale=a[:, t : t + 1],
                    )
                else:
                    eng.tensor_scalar(
                        d[:, t, :], src[:, t, :], a[:, t : t + 1],
                        mb[:, t : t + 1], op0=ALU.mult, op1=ALU.add,
                    )
            return d

        V, G, SC = nc.vector, nc.gpsimd, nc.scalar

        def emit_t(h_sb):
            """t = A h  (t[cy, j], fp8; copies on Scalar)."""
            t_sb = tpool.tile([P, CT, HW], F8, tag="t", name="t")
            for nh in range(IB):
                for mt in range(CT):
                    ps = pspool.tile([P, IBS], F32, tag="ps", name="ps")
                    for kp in range(2):
                        nc.tensor.matmul(
                            ps[:],
                            A_sb[:, kp, :, mt * P : (mt + 1) * P],
                            h_sb[:, 2 * kp : 2 * kp + 2, nh * IBS : (nh + 1) * IBS],
                            start=(kp == 0), stop=(kp == 1), perf_mode=DR,
                        )
                    nc.scalar.copy(t_sb[:, mt, nh * IBS : (nh + 1) * IBS], ps[:])
            return t_sb

        def emit_uT(h_sb):
            """uT = h^T Bm^T  (uT[j, co], fp8; copies split Scalar/DVE)."""
            uT_sb = upool.tile([P, JT, C], F8, tag="u", name="u")
            for jt in range(JT):
                ps = pspool.tile([P, C], F32, tag="ps", name="ps")
                for kp in range(2):
                    nc.tensor.matmul(
                        ps[:],
                        h_sb[:, 2 * kp : 2 * kp + 2, jt * P : (jt + 1) * P],
                        Bm_sb[:, kp, :, :],
                        start=(kp == 0), stop=(kp == 1), perf_mode=DR,
                    )
                if jt < 4:
                    nc.scalar.copy(uT_sb[:, jt, :], ps[:])
                else:
                    nc.vector.tensor_copy(uT_sb[:, jt, :], ps[:])
            return uT_sb

        def emit_bias(h_sb):
            """bq logit bias: r[j] = g^T h, bias = SM*r + EXPB."""
            rps = zpool.tile([P, JT], F32, tag="z", name="rb")
            for jt in range(JT):
                for kt in range(CT):
                    nc.tensor.matmul(
                        rps[:, jt : jt + 1],
                        h_sb[:, kt, jt * P : (jt + 1) * P],
                        g_sb[:, kt : kt + 1],
                        start=(kt == 0), stop=(kt == CT - 1),
                    )
            bias_sb = smpool.tile([P, JT], F32, tag="bia", name="bia")
            nc.vector.tensor_scalar(
                bias_sb[:], rps[:], SM, EXPB, op0=ALU.mult, op1=ALU.add
            )
            return bias_sb

        def emit_attention(b, t_sb, uT_sb, yn_sb, xres, bias_sb):
            outv = out[b].rearrange("p (t n) -> p t n", n=HW)
            e = [
                epool.tile([P, JT, IBS], F8, tag=f"e{ib}", name=f"e{ib}")
                for ib in range(IB)
            ]
            zps = {}

            def S_group(ib, jt):
                ps = pspool.tile([P, IBS], F32, tag="ps", name="ps")
                for kp in range(2):
                    nc.tensor.matmul(
                        ps[:],
                        t_sb[:, 2 * kp : 2 * kp + 2, jt * P : (jt + 1) * P],
                        yn_sb[:, 2 * kp : 2 * kp + 2, ib * IBS : (ib + 1) * IBS],
                        start=(kp == 0), stop=(kp == 1), perf_mode=DR,
                    )
                bias = bias_sb[:, jt : jt + 1] if has_bq else expb_sb[:]
                nc.scalar.activation(
                    e[ib][:, jt, :], ps[:], AF.Exp, bias=bias, scale=EXPS
                )

            def Z_mm(ib, pr):
                if pr == 0:
                    zps[ib] = zpool.tile([P, IBS], F32, tag="z", name="z")
                nc.tensor.matmul(
                    zps[ib][:], ones_sb[:],
                    e[ib][:, 2 * pr : 2 * pr + 2, :],
                    start=(pr == 0), stop=(pr == 3), perf_mode=DR,
                )

            def recip(ib):
                rz = rzpool.tile([P, IBS], F32, tag="rz", name="rz")
                nc.vector.reciprocal_approx_fast(rz[:], zps[ib][:])
                return rz

            def o_block(ib, rz, ot):
                isl = slice(ib * IBS, (ib + 1) * IBS)
                for ct in range(CT):
                    ops_ = opool.tile([P, IBS], F32, tag="o", name="o")
                    for pr in range(4):
                        nc.tensor.matmul(
                            ops_[:],
                            uT_sb[:, 2 * pr : 2 * pr + 2, ct * P : (ct + 1) * P],
                            e[ib][:, 2 * pr : 2 * pr + 2, :],
                            start=(pr == 0), stop=(pr == 3), perf_mode=DR,
                        )
                    om = ompool.tile([P, IBS], F32, tag="om", name="om")
                    nc.vector.tensor_tensor(om[:], ops_[:], rz[:], op=ALU.mult)
                    nc.gpsimd.tensor_tensor(
                        ot[:, ct, :], om[:], xres[:, ct, isl], op=ALU.add
                    )
                nc.sync.dma_start(outv[:, :, isl], ot[:])

            ot0 = outpool.tile([P, CT, IBS], F32, tag="ot", name="ot0")
            ot1 = outpool.tile([P, CT, IBS], F32, tag="ot", name="ot1")
            for jt in range(JT):
                S_group(0, jt)
            for pr in range(3):
                Z_mm(0, pr)
            for jt in range(4):
                S_group(1, jt)
            Z_mm(0, 3)
            rz0 = recip(0)
            o_block(0, rz0, ot0)
            for jt in range(4, JT):
                S_group(1, jt)
            for pr in range(3):
                Z_mm(1, pr)
            Z_mm(1, 3)
            rz1 = recip(1)
            o_block(1, rz1, ot1)

        def make_xres(x_sb):
            if not has_bpp:
                return x_sb
            xres = outpool.tile([P, CT, HW], F32, tag="xb", name="xb")
            for t in range(CT):
                nc.gpsimd.tensor_scalar(
                    xres[:, t, :], x_sb[:, t, :], prm_sb[:, 4, t : t + 1],
                    None, op0=ALU.add,
                )
            return xres

        # ---- all GroupNorm work is front-loaded (BPC=2): the x0 chain gates
        # the first matmul; everything else fills scheduler bubbles. All
        # Sqrts precede the first Exp so the activation table loads only
        # twice in the whole kernel.
        xy = [load_xy(b) for b in range(BPC)]
        sts = [stpool.tile([P, 2, CT, 3], F32, tag="st", name=f"st{b}")
               for b in range(BPC)]

        stats_pre(xy[0][0], sts[0], 0)
        ab = stats_post(stats_mm(sts[0], 0), 0, "x0")
        h0 = gn_apply(xy[0][0], hpool, "h", ab, [V, V, SC, G])
        t0 = emit_t(h0)
        stats_pre(xy[0][1], sts[0], 1)
        ab = stats_post(stats_mm(sts[0], 1), 1, "y0")
        yn0 = gn_apply(xy[0][1], ynpool, "yn", ab, [V, SC, G, G])
        u0 = emit_uT(h0)
        bias0 = emit_bias(h0) if has_bq else None

        hs, yns, ts, us, biases = [h0], [yn0], [t0], [u0], [bias0]
        for b in range(1, BPC):
            stats_pre(xy[b][0], sts[b], 0)
            ab = stats_post(stats_mm(sts[b], 0), 0, f"x{b}")
            hb = gn_apply(xy[b][0], hpool, "h", ab, [V, SC, G, G])
            stats_pre(xy[b][1], sts[b], 1)
            ab = stats_post(stats_mm(sts[b], 1), 1, f"y{b}")
            yns.append(gn_apply(xy[b][1], ynpool, "yn", ab, [V, SC, G, G]))
            hs.append(hb)

        for b in range(BPC):
            xres = make_xres(xy[b][0])
            if b > 0:
                ts.append(emit_t(hs[b]))
                us.append(emit_uT(hs[b]))
                biases.append(emit_bias(hs[b]) if has_bq else None)
            emit_attention(b, ts[b], us[b], yns[b], xres, biases[b])


_CACHE = {}


def _build(has_bq, has_bpp):
    key = ("nc", has_bq, has_bpp)
    if key in _CACHE:
        return _CACHE[key]
    nc = bacc.Bacc("TRN2", target_bir_lowering=False, debug=False)
    aps = {
        "xs": nc.dram_tensor("xs", [BPC, P, CT * HW], U16, kind="ExternalInput").ap(),
        "ys": nc.dram_tensor("ys", [BPC, P, CT * HW], U16, kind="ExternalInput").ap(),
        "A": nc.dram_tensor("A", [P, 4 * C], U8, kind="ExternalInput").ap(),
        "Bm": nc.dram_tensor("Bm", [P, 4 * C], U8, kind="ExternalInput").ap(),
        "prm": nc.dram_tensor("prm", [P, 5 * CT], F32, kind="ExternalInput").ap(),
        "amat": nc.dram_tensor("amat", [P, P], F32, kind="ExternalInput").ap(),
        "out": nc.dram_tensor("out", [BPC, P, CT * HW], F32, kind="ExternalOutput").ap(),
    }
    if has_bq:
        aps["gv"] = nc.dram_tensor("gv", [P, CT], U8, kind="ExternalInput").ap()
    with tile.TileContext(nc) as tc:
        _emit(tc, aps, has_bq, has_bpp)
    nc.compile()
    _CACHE[key] = nc
    return nc


def _pack_chw(a):
    """[*, C, HW] -> [*, P, CT*HW] matching SBUF layout c = t*128 + p."""
    lead = a.shape[:-2]
    a = a.reshape(*lead, CT, P, HW)
    a = np.moveaxis(a, -3, -2)          # [..., P, CT, HW]
    return np.ascontiguousarray(a.reshape(*lead, P, CT * HW))


def _unpack_chw(a):
    """[*, P, CT*HW] -> [*, C, HW]."""
    lead = a.shape[:-2]
    a = a.reshape(*lead, P, CT, HW)
    a = np.moveaxis(a, -2, -3)          # [..., CT, P, HW]
    return np.ascontiguousarray(a.reshape(*lead, CT * P, HW))


def _q8(a):
    return np.clip(a, -240.0, 240.0).astype(NPF8)


def _pack_w(wT, scale):
    """wT [cin, cout] -> fp8 bytes [P, 2*2*C]: [p, kpair, ktile2, cout],
    cin = (2*kpair + ktile2)*128 + p."""
    w8 = _q8(wT * scale).view(np.uint8)
    w8 = w8.reshape(2, 2, P, C).transpose(2, 0, 1, 3)
    return np.ascontiguousarray(w8.reshape(P, 4 * C))


def _host_inputs(x, y, norm_scale, norm_bias, norm1_scale, norm1_bias,
                 wq, bq, wk, bk, wv, bv, wp, bp):
    f = lambda a: np.ascontiguousarray(np.asarray(a, dtype=np.float32))
    x = f(x).reshape(B, C, HW)
    y = f(y).reshape(B, C, HW)
    wq, wk, wv, wp = f(wq), f(wk), f(wv), f(wp)
    A = wq.T @ wk                       # [cy, ch]
    Bm = wp @ wv                        # [co, ci]
    # bk cancels in softmax; bv folds into bp' because softmax rows sum to 1
    bpp = f(bp) + wp @ f(bv)
    # rows: [gamma_x, gamma_y, beta_x, beta_y, bpp]
    prm = np.stack([f(norm_scale), f(norm1_scale), f(norm_bias), f(norm1_bias),
                    bpp]).astype(np.float32)
    prm = np.ascontiguousarray(
        prm.reshape(5, CT, P).transpose(2, 0, 1).reshape(P, 5 * CT)
    )
    amat = np.zeros((P, P), np.float32)
    for g in range(P // GSIZE):
        amat[g * GSIZE : (g + 1) * GSIZE, g * GSIZE : (g + 1) * GSIZE] = 1.0 / GSIZE
    has_bq = bool(np.any(np.asarray(bq)))
    has_bpp = bool(np.any(bpp))
    shared = {
        "A": _pack_w(A.T, SA),          # lhsT[cin=ch, cout=cy]
        "Bm": _pack_w(Bm.T, SB),        # rhs[cin=ci, cout=co]
        "prm": prm, "amat": amat,
    }
    if has_bq:
        gv = wk.T @ f(bq)               # [ci]
        gv8 = _q8(gv).view(np.uint8).reshape(CT, P).T
        shared["gv"] = np.ascontiguousarray(gv8)

    xb = _pack_chw(x.astype(NPBF16).view(np.uint16))
    yb = _pack_chw(y.astype(NPBF16).view(np.uint16))
    in_maps = []
    for core in range(NCORES):
        sl = slice(core * BPC, (core + 1) * BPC)
        in_maps.append({"xs": xb[sl], "ys": yb[sl], **shared})
    return in_maps, has_bq, has_bpp


def _run(in_maps, has_bq, has_bpp, trace=False):
    nc = _build(has_bq, has_bpp)
    res = run_bass_kernel_spmd(
        nc, in_maps, core_ids=list(range(NCORES)), trace=trace
    )
    out = np.concatenate(
        [_unpack_chw(res.results[i]["out"]) for i in range(NCORES)], axis=0
    ).reshape(B, C, H, W)
    return out, res


def kernel(**inputs):
    in_maps, has_bq, has_bpp = _host_inputs(**inputs)
    out, _ = _run(in_maps, has_bq, has_bpp, trace=False)
    return out


# revision 25
# speedup vs baseline: 1.9061x; 1.0158x over previous
"""AttnBlock fusion kernel for Trainium2 (Bass/Tile), 8 NeuronCores.

Reference computation (per batch element b; c=512 channels, hw=1024 spatial):
    h  = GroupNorm(32, c)(x) ; k = Wk h + bk ; v = Wv h + bv
    y_ = GroupNorm(32, c)(y) ; q = Wq y_ + bq
    attn = softmax_j(q^T k / sqrt(c)) ; o = v @ attn^T ; out = x + Wp o + bp

Sharding: pure data parallel over batch (16 batches / 8 cores = 2 each).

Algebraic folds (host side, exact):
  * S = q^T k = y_^T (Wq^T Wk) h  -> A := Wq^T Wk precomputed; the q and k
    projections disappear (one matmul t = A h replaces both).
  * Wp (v @ P) = (Wp Wv) h @ P    -> Bm := Wp Wv precomputed; the v and
    proj_out projections disappear (u = Bm h replaces both).
  * bk adds a per-i constant to logits -> cancels in softmax.
  * bv contributes Wp bv exactly (softmax rows sum to 1) -> bp' = bp + Wp bv.
  * bq (zero in practice) handled by a compiled-in logit-bias path.

Precision scheme (validated vs reference: rel_l2 ~ 5.6e-3, gate 2e-2):
  * All five big matmuls run fp8(e4m3) with MatmulPerfMode.DoubleRow:
    2 contraction tiles per pass = 2x throughput over fp32r/bf16.
  * A, Bm scaled by 16 so t = A h and u = Bm h land in e4m3 range (+-240);
    1/16 is folded into the exp scale (t side) and into ones=16 for the
    Z row-sum matmul (u side, via 1/Z).
  * E = exp(s S - 3): the -3 shift is softmax-invariant and keeps
    max(E) ~ 31 < 240 so no fp8 overflow-to-inf.
  * x, y stored bf16 on chip (stats + gn + residual), accumulation fp32.

Engine split per batch (PE ~28us is the bound):
  PE      t/uT/S/Z/o DoubleRow matmuls + tiny GN aggregation matmul
  Scalar  exp(S), t PSUM->fp8 copies, GN sqrt
  DVE     bn_stats GN statistics, uT copies, 1/Z (fast approx), o*(1/Z)
  GpSimd  GN apply (bf16->fp8), residual add to output
"""

import math
import os
import sys
from contextlib import ExitStack

import numpy as np
import ml_dtypes

for _p in ("/opt/trn_rl_repo", "/root/.axon_site/_ro/trn_rl_repo"):
    if os.path.isdir(_p) and _p not in sys.path:
        sys.path.append(_p)

import concourse.bass as bass
import concourse.bacc as bacc
import concourse.mybir as mybir
import concourse.tile as tile
from concourse.bass_utils import run_bass_kernel_spmd

F32 = mybir.dt.float32
BF16 = mybir.dt.bfloat16
F8 = mybir.dt.float8e4
U8 = mybir.dt.uint8
U16 = mybir.dt.uint16
AF = mybir.ActivationFunctionType
ALU = mybir.AluOpType
DR = mybir.MatmulPerfMode.DoubleRow

B, C, H, W = 16, 512, 32, 32
HW = H * W                  # 1024
NCORES = 8
BPC = B // NCORES           # 2 batches per core
P = 128                     # SBUF partitions
CT = C // P                 # 4 channel tiles
JT = HW // P                # 8 key-position tiles
IBS = 512                   # query positions per i-block
IB = HW // IBS              # 2 i-blocks
GROUPS = 32
GSIZE = C // GROUPS         # 16 channels per group
EPS = 1e-6
SM = float(C) ** -0.5
SA = 16.0                   # scale folded into A
SB = 16.0                   # scale folded into Bm (and into ones for Z)
EXPS = SM / SA
EXPB = -3.0                 # softmax-invariant logit shift, keeps E < 240

NPF8 = ml_dtypes.float8_e4m3   # IEEE e4m3 (bias 7, max 240) == TRN FP8_EXP4
NPBF16 = ml_dtypes.bfloat16


def _emit(tc, aps, has_bq, has_bpp, id_aff):
    nc = tc.nc
    xs, ys, out = aps["xs"], aps["ys"], aps["out"]

    with ExitStack() as ctx:
        cpool = ctx.enter_context(tc.tile_pool(name="const", bufs=1))
        wpool = ctx.enter_context(tc.tile_pool(name="w", bufs=1))
        xpool = ctx.enter_context(tc.tile_pool(name="xin", bufs=2))
        ypool = ctx.enter_context(tc.tile_pool(name="yin", bufs=2))
        hpool = ctx.enter_context(tc.tile_pool(name="hb", bufs=2))
        ynpool = ctx.enter_context(tc.tile_pool(name="ynb", bufs=2))
        tpool = ctx.enter_context(tc.tile_pool(name="tb", bufs=2))
        upool = ctx.enter_context(tc.tile_pool(name="ub", bufs=2))
        epool = ctx.enter_context(tc.tile_pool(name="eb", bufs=2))
        stpool = ctx.enter_context(tc.tile_pool(name="st", bufs=2))
        smpool = ctx.enter_context(tc.tile_pool(name="sm", bufs=3))
        ompool = ctx.enter_context(tc.tile_pool(name="om", bufs=3))
        outpool = ctx.enter_context(tc.tile_pool(name="outb", bufs=2))
        rzpool = ctx.enter_context(tc.tile_pool(name="rz", bufs=2))
        pspool = ctx.enter_context(tc.tile_pool(name="ps", bufs=3, space="PSUM"))
        zpool = ctx.enter_context(tc.tile_pool(name="z", bufs=1, space="PSUM"))
        opool = ctx.enter_context(tc.tile_pool(name="o", bufs=4, space="PSUM"))

        # ---- constants / weights ----
        prm_sb = cpool.tile([P, 5, CT], F32)
        nc.gpsimd.dma_start(prm_sb[:], aps["prm"].rearrange("p (q t) -> p q t", t=CT))
        amat_sb = cpool.tile([P, P], F32)
        nc.gpsimd.dma_start(amat_sb[:], aps["amat"][:])
        ones_sb = cpool.tile([P, 2, P], F8)
        nc.vector.memset(ones_sb[:], SB)
        expb_sb = cpool.tile([P, 1], F32)
        nc.vector.memset(expb_sb[:], EXPB)
        A_sb = wpool.tile([P, 2, 2, C], F8)
        nc.gpsimd.dma_start(
            A_sb[:], aps["A"].rearrange("p (a b o) -> p a b o", a=2, b=2).bitcast(F8)
        )
        Bm_sb = wpool.tile([P, 2, 2, C], F8)
        nc.gpsimd.dma_start(
            Bm_sb[:], aps["Bm"].rearrange("p (a b o) -> p a b o", a=2, b=2).bitcast(F8)
        )
        if has_bq:
            g_sb = cpool.tile([P, CT], F8)
            nc.gpsimd.dma_start(g_sb[:], aps["gv"].bitcast(F8))

        def load_xy(b):
            """Split DMAs so stats can start before the full tensor lands;
            x0 per-tile since its first tile gates the whole pipeline."""
            x_sb = xpool.tile([P, CT, HW], BF16, tag="x")
            v = xs[b].rearrange("p (t n) -> p t n", n=HW).bitcast(BF16)
            step = 1 if b == 0 else 2
            for c0 in range(0, CT, step):
                nc.sync.dma_start(x_sb[:, c0 : c0 + step, :],
                                  v[:, c0 : c0 + step, :])
            y_sb = ypool.tile([P, CT, HW], BF16, tag="y")
            v = ys[b].rearrange("p (t n) -> p t n", n=HW).bitcast(BF16)
            for hf in range(2):
                nc.sync.dma_start(y_sb[:, 2 * hf : 2 * hf + 2, :],
                                  v[:, 2 * hf : 2 * hf + 2, :])
            return x_sb, y_sb

        def stats_pre(src, st, u):
            """DVE bn_stats: per-channel [mean, var, mean^2] -> st[:, u]."""
            for t in range(CT):
                bns = smpool.tile([P, 2, 6], F32, tag="bns")
                for h2 in range(2):
                    nc.vector.bn_stats(
                        bns[:, h2, :], src[:, t, h2 * 512 : (h2 + 1) * 512]
                    )
                nc.vector.bn_aggr(st[:, u, t, 0:2], bns[:])
            nc.vector.tensor_tensor(
                st[:, u, :, 2], st[:, u, :, 0], st[:, u, :, 0], op=ALU.mult
            )

        def stats_mm(st, u):
            """PE: per-group averaging of [mean, var, mean^2] for one tensor."""
            gt = pspool.tile([P, IBS], F32, tag="ps", name="gps")
            gps = gt[:, 0 : CT * 3]
            nc.tensor.matmul(gps, amat_sb[:], st[:, u], start=True, stop=True)
            return gps

        def stats_post(gps, u, uid):
            """a = rstd*gamma, mb = beta - mean*a for one tensor: [P, CT].
            rstd = 1/(sqrt on ACT, accurate reciprocal on DVE); chain kept
            short because each serial hop pays a scheduler-interleave delay."""
            g = smpool.tile([P, CT, 3], F32, tag=f"g{uid}")
            nc.vector.tensor_copy(g[:], gps)
            # var_g = E[var] + E[mean^2] - E[mean]^2 (equal-count partitions)
            msq = smpool.tile([P, CT], F32, tag=f"ms{uid}")
            nc.vector.tensor_tensor(msq[:], g[:, :, 0], g[:, :, 0], op=ALU.mult)
            var = smpool.tile([P, CT], F32, tag=f"va{uid}")
            nc.vector.scalar_tensor_tensor(
                var[:], g[:, :, 1], EPS, g[:, :, 2], op0=ALU.add, op1=ALU.add
            )
            nc.vector.tensor_tensor(var[:], var[:], msq[:], op=ALU.subtract)
            std = smpool.tile([P, CT], F32, tag=f"sd{uid}")
            nc.scalar.activation(std[:], var[:], AF.Sqrt)
            r0 = smpool.tile([P, CT], F32, tag=f"r0{uid}")
            nc.vector.reciprocal(r0[:], std[:])
            mb = smpool.tile([P, CT], F32, tag=f"mb{uid}")
            if id_aff:
                # gamma == 1, beta == 0: a = rstd, mb = -mean*rstd
                nc.vector.scalar_tensor_tensor(
                    mb[:], g[:, :, 0], -1.0, r0[:], op0=ALU.mult, op1=ALU.mult
                )
                return r0, mb
            a = smpool.tile([P, CT], F32, tag=f"a{uid}")
            nc.vector.tensor_tensor(a[:], r0[:], prm_sb[:, u, :], op=ALU.mult)
            nc.vector.tensor_tensor(mb[:], g[:, :, 0], a[:], op=ALU.mult)
            nc.vector.tensor_tensor(mb[:], prm_sb[:, 2 + u, :], mb[:], op=ALU.subtract)
            return a, mb

        def gn_apply(src, pool, tag, ab, engines):
            """Per-tile affine-normalize, tile t on engines[t] (bf16 -> fp8)."""
            a, mb = ab
            d = pool.tile([P, CT, HW], F8, tag=tag)
            for t in range(CT):
                eng = engines[t]
                if eng is nc.scalar:
                    nc.scalar.activation(
                        d[:, t, :], src[:, t, :], AF.Identity,
                        bias=mb[:, t : t + 1], scale=a[:, t : t + 1],
                    )
                else:
                    eng.tensor_scalar(
                        d[:, t, :], src[:, t, :], a[:, t : t + 1],
                        mb[:, t : t + 1], op0=ALU.mult, op1=ALU.add,
                    )
            return d

        V, G, SC = nc.vector, nc.gpsimd, nc.scalar

        def emit_t(h_sb):
            """t = A h  (t[cy, j], fp8; copies on Scalar)."""
            t_sb = tpool.tile([P, CT, HW], F8, tag="t", name="t")
            for nh in range(IB):
                for mt in range(CT):
                    ps = pspool.tile([P, IBS], F32, tag="ps", name="ps")
                    for kp in range(2):
                        nc.tensor.matmul(
                            ps[:],
                            A_sb[:, kp, :, mt * P : (mt + 1) * P],
                            h_sb[:, 2 * kp : 2 * kp + 2, nh * IBS : (nh + 1) * IBS],
                            start=(kp == 0), stop=(kp == 1), perf_mode=DR,
                        )
                    nc.scalar.copy(t_sb[:, mt, nh * IBS : (nh + 1) * IBS], ps[:])
            return t_sb

        def emit_uT(h_sb):
            """uT = h^T Bm^T  (uT[j, co], fp8; copies split Scalar/DVE)."""
            uT_sb = upool.tile([P, JT, C], F8, tag="u", name="u")
            for jt in range(JT):
                ps = pspool.tile([P, C], F32, tag="ps", name="ps")
                for kp in range(2):
                    nc.tensor.matmul(
                        ps[:],
                        h_sb[:, 2 * kp : 2 * kp + 2, jt * P : (jt + 1) * P],
                        Bm_sb[:, kp, :, :],
                        start=(kp == 0), stop=(kp == 1), perf_mode=DR,
                    )
                if jt < 4:
                    nc.scalar.copy(uT_sb[:, jt, :], ps[:])
                else:
                    nc.vector.tensor_copy(uT_sb[:, jt, :], ps[:])
            return uT_sb

        def emit_bias(h_sb):
            """bq logit bias: r[j] = g^T h, bias = SM*r + EXPB."""
            rps = zpool.tile([P, JT], F32, tag="z", name="rb")
            for jt in range(JT):
                for kt in range(CT):
                    nc.tensor.matmul(
                        rps[:, jt : jt + 1],
                        h_sb[:, kt, jt * P : (jt + 1) * P],
                        g_sb[:, kt : kt + 1],
                        start=(kt == 0), stop=(kt == CT - 1),
                    )
            bias_sb = smpool.tile([P, JT], F32, tag="bia", name="bia")
            nc.vector.tensor_scalar(
                bias_sb[:], rps[:], SM, EXPB, op0=ALU.mult, op1=ALU.add
            )
            return bias_sb

        def emit_attention(b, t_sb, uT_sb, yn_sb, xres, bias_sb):
            outv = out[b].rearrange("p (t n) -> p t n", n=HW)
            e = [
                epool.tile([P, JT, IBS], F8, tag=f"e{ib}", name=f"e{ib}")
                for ib in range(IB)
            ]
            zps = {}

            def S_group(ib, jt):
                ps = pspool.tile([P, IBS], F32, tag="ps", name="ps")
                for kp in range(2):
                    nc.tensor.matmul(
                        ps[:],
                        t_sb[:, 2 * kp : 2 * kp + 2, jt * P : (jt + 1) * P],
                        yn_sb[:, 2 * kp : 2 * kp + 2, ib * IBS : (ib + 1) * IBS],
                        start=(kp == 0), stop=(kp == 1), perf_mode=DR,
                    )
                bias = bias_sb[:, jt : jt + 1] if has_bq else expb_sb[:]
                nc.scalar.activation(
                    e[ib][:, jt, :], ps[:], AF.Exp, bias=bias, scale=EXPS
                )

            def Z_mm(ib, pr):
                if pr == 0:
                    zps[ib] = zpool.tile([P, IBS], F32, tag="z", name="z")
                nc.tensor.matmul(
                    zps[ib][:], ones_sb[:],
                    e[ib][:, 2 * pr : 2 * pr + 2, :],
                    start=(pr == 0), stop=(pr == 3), perf_mode=DR,
                )

            def recip(ib):
                rz = rzpool.tile([P, IBS], F32, tag="rz", name="rz")
                nc.vector.reciprocal_approx_fast(rz[:], zps[ib][:])
                return rz

            def o_block(ib, rz, ot, last=False):
                isl = slice(ib * IBS, (ib + 1) * IBS)
                for ct in range(CT):
                    ops_ = opool.tile([P, IBS], F32, tag="o", name="o")
                    for pr in range(4):
                        nc.tensor.matmul(
                            ops_[:],
                            uT_sb[:, 2 * pr : 2 * pr + 2, ct * P : (ct + 1) * P],
                            e[ib][:, 2 * pr : 2 * pr + 2, :],
                            start=(pr == 0), stop=(pr == 3), perf_mode=DR,
                        )
                    om = ompool.tile([P, IBS], F32, tag="om", name="om")
                    nc.vector.tensor_tensor(om[:], ops_[:], rz[:], op=ALU.mult)
                    # final block: drain on DVE + per-ct DMA for shortest tail
                    eng = nc.vector if last else nc.gpsimd
                    eng.tensor_tensor(
                        ot[:, ct, :], om[:], xres[:, ct, isl], op=ALU.add
                    )
                    if last:
                        nc.sync.dma_start(outv[:, ct, isl], ot[:, ct, :])
                if not last:
                    nc.sync.dma_start(outv[:, :, isl], ot[:])

            ot0 = outpool.tile([P, CT, IBS], F32, tag="ot", name="ot0")
            ot1 = outpool.tile([P, CT, IBS], F32, tag="ot", name="ot1")
            for jt in range(JT):
                S_group(0, jt)
            for pr in range(3):
                Z_mm(0, pr)
            for jt in range(4):
                S_group(1, jt)
            Z_mm(0, 3)
            rz0 = recip(0)
            o_block(0, rz0, ot0)
            for jt in range(4, JT):
                S_group(1, jt)
            for pr in range(3):
                Z_mm(1, pr)
            Z_mm(1, 3)
            rz1 = recip(1)
            o_block(1, rz1, ot1, last=(b == BPC - 1))

        def make_xres(x_sb):
            if not has_bpp:
                return x_sb
            xres = outpool.tile([P, CT, HW], F32, tag="xb", name="xb")
            for t in range(CT):
                nc.gpsimd.tensor_scalar(
                    xres[:, t, :], x_sb[:, t, :], prm_sb[:, 4, t : t + 1],
                    None, op0=ALU.add,
                )
            return xres

        # ---- all GroupNorm work is front-loaded (BPC=2): the x0 chain gates
        # the first matmul; everything else fills scheduler bubbles. All
        # Sqrts precede the first Exp so the activation table loads only
        # twice in the whole kernel.
        xy = [load_xy(b) for b in range(BPC)]
        sts = [stpool.tile([P, 2, CT, 3], F32, tag="st", name=f"st{b}")
               for b in range(BPC)]

        stats_pre(xy[0][0], sts[0], 0)
        ab = stats_post(stats_mm(sts[0], 0), 0, "x0")
        h0 = gn_apply(xy[0][0], hpool, "h", ab, [V, V, SC, G])
        t0 = emit_t(h0)
        stats_pre(xy[0][1], sts[0], 1)
        ab = stats_post(stats_mm(sts[0], 1), 1, "y0")
        yn0 = gn_apply(xy[0][1], ynpool, "yn", ab, [V, SC, G, G])
        u0 = emit_uT(h0)
        bias0 = emit_bias(h0) if has_bq else None

        hs, yns, ts, us, biases = [h0], [yn0], [t0], [u0], [bias0]
        for b in range(1, BPC):
            stats_pre(xy[b][0], sts[b], 0)
            ab = stats_post(stats_mm(sts[b], 0), 0, f"x{b}")
            hb = gn_apply(xy[b][0], hpool, "h", ab, [V, SC, G, G])
            stats_pre(xy[b][1], sts[b], 1)
            ab = stats_post(stats_mm(sts[b], 1), 1, f"y{b}")
            yns.append(gn_apply(xy[b][1], ynpool, "yn", ab, [V, SC, G, G]))
            hs.append(hb)

        for b in range(BPC):
            xres = make_xres(xy[b][0])
            if b > 0:
                ts.append(emit_t(hs[b]))
                us.append(emit_uT(hs[b]))
                biases.append(emit_bias(hs[b]) if has_bq else None)
            emit_attention(b, ts[b], us[b], yns[b], xres, biases[b])


_CACHE = {}


def _build(has_bq, has_bpp, id_aff):
    key = ("nc", has_bq, has_bpp, id_aff)
    if key in _CACHE:
        return _CACHE[key]
    nc = bacc.Bacc("TRN2", target_bir_lowering=False, debug=False)
    aps = {
        "xs": nc.dram_tensor("xs", [BPC, P, CT * HW], U16, kind="ExternalInput").ap(),
        "ys": nc.dram_tensor("ys", [BPC, P, CT * HW], U16, kind="ExternalInput").ap(),
        "A": nc.dram_tensor("A", [P, 4 * C], U8, kind="ExternalInput").ap(),
        "Bm": nc.dram_tensor("Bm", [P, 4 * C], U8, kind="ExternalInput").ap(),
        "prm": nc.dram_tensor("prm", [P, 5 * CT], F32, kind="ExternalInput").ap(),
        "amat": nc.dram_tensor("amat", [P, P], F32, kind="ExternalInput").ap(),
        "out": nc.dram_tensor("out", [BPC, P, CT * HW], F32, kind="ExternalOutput").ap(),
    }
    if has_bq:
        aps["gv"] = nc.dram_tensor("gv", [P, CT], U8, kind="ExternalInput").ap()
    with tile.TileContext(nc) as tc:
        _emit(tc, aps, has_bq, has_bpp, id_aff)
    nc.compile()
    _CACHE[key] = nc
    return nc


def _pack_chw(a):
    """[*, C, HW] -> [*, P, CT*HW] matching SBUF layout c = t*128 + p."""
    lead = a.shape[:-2]
    a = a.reshape(*lead, CT, P, HW)
    a = np.moveaxis(a, -3, -2)          # [..., P, CT, HW]
    return np.ascontiguousarray(a.reshape(*lead, P, CT * HW))


def _unpack_chw(a):
    """[*, P, CT*HW] -> [*, C, HW]."""
    lead = a.shape[:-2]
    a = a.reshape(*lead, P, CT, HW)
    a = np.moveaxis(a, -2, -3)          # [..., CT, P, HW]
    return np.ascontiguousarray(a.reshape(*lead, CT * P, HW))


def _q8(a):
    return np.clip(a, -240.0, 240.0).astype(NPF8)


def _pack_w(wT, scale):
    """wT [cin, cout] -> fp8 bytes [P, 2*2*C]: [p, kpair, ktile2, cout],
    cin = (2*kpair + ktile2)*128 + p."""
    w8 = _q8(wT * scale).view(np.uint8)
    w8 = w8.reshape(2, 2, P, C).transpose(2, 0, 1, 3)
    return np.ascontiguousarray(w8.reshape(P, 4 * C))


def _host_inputs(x, y, norm_scale, norm_bias, norm1_scale, norm1_bias,
                 wq, bq, wk, bk, wv, bv, wp, bp):
    f = lambda a: np.ascontiguousarray(np.asarray(a, dtype=np.float32))
    x = f(x).reshape(B, C, HW)
    y = f(y).reshape(B, C, HW)
    wq, wk, wv, wp = f(wq), f(wk), f(wv), f(wp)
    A = wq.T @ wk                       # [cy, ch]
    Bm = wp @ wv                        # [co, ci]
    # bk cancels in softmax; bv folds into bp' because softmax rows sum to 1
    bpp = f(bp) + wp @ f(bv)
    # rows: [gamma_x, gamma_y, beta_x, beta_y, bpp]
    prm = np.stack([f(norm_scale), f(norm1_scale), f(norm_bias), f(norm1_bias),
                    bpp]).astype(np.float32)
    prm = np.ascontiguousarray(
        prm.reshape(5, CT, P).transpose(2, 0, 1).reshape(P, 5 * CT)
    )
    amat = np.zeros((P, P), np.float32)
    for g in range(P // GSIZE):
        amat[g * GSIZE : (g + 1) * GSIZE, g * GSIZE : (g + 1) * GSIZE] = 1.0 / GSIZE
    has_bq = bool(np.any(np.asarray(bq)))
    has_bpp = bool(np.any(bpp))
    id_aff = bool(
        np.all(prm[:, 0 * CT : 2 * CT] == 1.0) and
        np.all(prm[:, 2 * CT : 4 * CT] == 0.0)
    )
    shared = {
        "A": _pack_w(A.T, SA),          # lhsT[cin=ch, cout=cy]
        "Bm": _pack_w(Bm.T, SB),        # rhs[cin=ci, cout=co]
        "prm": prm, "amat": amat,
    }
    if has_bq:
        gv = wk.T @ f(bq)               # [ci]
        gv8 = _q8(gv).view(np.uint8).reshape(CT, P).T
        shared["gv"] = np.ascontiguousarray(gv8)

    xb = _pack_chw(x.astype(NPBF16).view(np.uint16))
    yb = _pack_chw(y.astype(NPBF16).view(np.uint16))
    in_maps = []
    for core in range(NCORES):
        sl = slice(core * BPC, (core + 1) * BPC)
        in_maps.append({"xs": xb[sl], "ys": yb[sl], **shared})
    return in_maps, (has_bq, has_bpp, id_aff)


def _run(in_maps, flags, trace=False):
    nc = _build(*flags)
    res = run_bass_kernel_spmd(
        nc, in_maps, core_ids=list(range(NCORES)), trace=trace
    )
    out = np.concatenate(
        [_unpack_chw(res.results[i]["out"]) for i in range(NCORES)], axis=0
    ).reshape(B, C, H, W)
    return out, res


def kernel(**inputs):
    in_maps, flags = _host_inputs(**inputs)
    out, _ = _run(in_maps, flags, trace=False)
    return out
